# revision 2
# baseline (speedup 1.0000x reference)
"""Trainium2 Bass kernel for nn_MfdFC (spherical weighted-Frechet-mean layer).

Math (per row n of N=B*L=1024, all fp32):
  w = col-softmax(w_raw);  X = x[n] (64 points on S^63)
  a(o) <- x0;  3 iterations of:
      D[o,i] = <a_o, x_i>;  f = arccos(D)/sqrt(1-D^2)  (computed via the
      half-angle arctan identity + a custom-DVE quake rsqrt)
      S = w^T * f;  G = S @ X
      coefA[o] = sum_d A*G ; gn2 = sum G^2 - coefA^2   (exact identities)
      a_o <- (cos gn - sinc(gn)*coefA) * a_o + sinc(gn) * G_o
Sharding: data-parallel over rows; core k owns rows [128k, 128(k+1)).

Layout per core: 8 groups x 16 rows. Row r of a group lives at
partition-half r//8 (offset 64*(r//8)) and free-block r%8 (offset 64*(r%8))
of (128 x 512) group tiles. PE does per-row 64^3 matmuls; ACT holds the
single `trig_and_small` table set (Square/Arctan/Sin/Copy); GPSIMD takes
SBUF-only elementwise ops; DVE runs the custom fused ops and reductions.
"""
import math
import numpy as np

f32 = np.float32
FP = None  # mybir.dt.float32, set at import of concourse below

# ---------------------------------------------------------------------------
# constants
C_IN = 64
C_OUT = 64
D_DIM = 64
ROWS_PER_CORE = 128
N_CORES = 8
GROUP_ROWS = 16
N_GROUPS = ROWS_PER_CORE // GROUP_ROWS  # 8
CLIP = float(f32(1.0) - f32(2.0) ** -23)  # 0.99999988
RSQ_C1 = 1.7584694439735017e-30
RSQ_C2 = -2.755803843779718e-20
SHIFT1 = float(np.int32(1).view(f32))  # denormal whose bit pattern is 1
PI = float(f32(math.pi))
HALF_PI = float(f32(math.pi / 2.0))

_COMPILED = {}

# ---------------------------------------------------------------------------
# custom DVE ops

def _register_custom_ops():
    import concourse.dve_ops as dve_ops
    from concourse.dve_ops import DveOp
    from concourse.dve_spec import (
        Spec, Src0, Src1, C0, C1, C2, Zero, One, Bin, AluOp, lower, maxx,
        _has_src1 as has_src1,
    )
    from concourse.dve_uop import DveOpSpec
    from concourse.dve_table_gen import dve_ver_for

    if "ANT_RSQ_F" in dve_ops._SUB_OPCODE_FOR_NAME:
        return {n: op for n, op in ((o.name, o) for o in dve_ops.OPS)
                if n.startswith("ANT_")}

    def _ref_rsq_f(in0, in1, s0, s1, imm2):
        # in0 = u, in1 = float-view of ~(bits(u)>>1) (computed on GPSIMD)
        u = np.asarray(in0, f32)
        nt = np.asarray(in1, f32)
        m1 = (nt * f32(s0)).astype(f32)
        m2 = (m1 * nt).astype(f32)
        m3 = (m2 * f32(s0)).astype(f32)
        t = (m3 * u).astype(f32)
        return ((t + f32(s1)) * nt).astype(f32)

    _m1 = Src1 * C0
    _m3 = (_m1 * Src1) * C0
    RSQ_F = DveOp("ANT_RSQ_F",
                  Spec(body=((_m3 * Src0) + C1) * Src1, reference=_ref_rsq_f),
                  subdim=False, uops_sha={})

    def _ref_rsq_nr(in0, in1, s0, s1, imm2):
        u = np.asarray(in0, f32); y = np.asarray(in1, f32)
        a = (u * y).astype(f32)
        b = (a * y).astype(f32)
        return ((f32(s0) - (b * f32(s1)).astype(f32)) * y).astype(f32)

    RSQ_NR = DveOp("ANT_RSQ_NR",
                   Spec(body=(C0 - ((Src0 * Src1) * Src1) * C1) * Src1,
                        reference=_ref_rsq_nr),
                   subdim=False, uops_sha={})

    def _ref_zs(in0, in1, s0, s1, imm2):
        D = np.asarray(in0, f32); r = np.asarray(in1, f32)
        lt = (D < 0).astype(f32)
        return (((f32(1.0) + f32(s1) * lt).astype(f32) - D) * r).astype(f32)

    ZS_F = DveOp("ANT_ZS_F",
                 Spec(body=((One + (Src0 < Zero) * C1) - Src0) * Src1,
                      reference=_ref_zs),
                 subdim=False, uops_sha={})

    def _ref_ff(in0, in1, s0, s1, imm2):
        th = np.asarray(in0, f32); r = np.asarray(in1, f32)
        lt = (th < 0).astype(f32)
        return (((f32(s0) * lt).astype(f32) + (f32(s1) * th).astype(f32)) * r).astype(f32)

    F_F = DveOp("ANT_F_F",
                Spec(body=(((Src0 < Zero) * C0) + (Src0 * C1)) * Src1,
                     reference=_ref_ff),
                subdim=False, uops_sha={})

    def _ref_gn2(in0, in1, s0, s1, imm2):
        raw = np.asarray(in0, f32); c = np.asarray(in1, f32)
        return np.maximum((raw - (c * c).astype(f32)).astype(f32), f32(s0))

    GN2_F = DveOp("ANT_GN2_F",
                  Spec(body=maxx(Src0 - Src1 * Src1, C0), reference=_ref_gn2),
                  subdim=False, uops_sha={})

    ops = [RSQ_F, RSQ_NR, ZS_F, F_F, GN2_F]
    base = dve_ops._CUSTOM_DVE_ROW_BASE + len(dve_ops.OPS)
    for i, op in enumerate(ops):
        dve_ops._SUB_OPCODE_FOR_NAME[op.name] = base + i
    # pin shas by compiling once
    for trn in ("TRN2",):
        ver = dve_ver_for(trn)
        for op in ops:
            uops = lower(op.spec, ver=ver)
            s = DveOpSpec(name=op.name, opcode=dve_ops.get_dve_sub_opcode(op.name),
                          uops=uops, rd1_en=has_src1(op.spec))
            op.uops_sha[ver] = s.sha(ver)
    dve_ops.OPS.extend(ops)
    dve_ops.CUSTOM_DVE_SPECS.update({op.name: op.spec for op in ops})
    return {op.name: op for op in ops}


# ---------------------------------------------------------------------------
# per-core Bass program

def _row_slices(r):
    p = 64 * (r // 8)
    fb = 64 * (r % 8)
    return p, fb


def build_program(repeat=1, bufs=None, INTERLEAVE=2, rpg=GROUP_ROWS):
    global FP
    B = {"xg": 2, "work": 2, "ab": 3, "red": 2, "ps": 2}
    if bufs:
        B.update(bufs)
    n_groups = ROWS_PER_CORE // rpg
    from contextlib import ExitStack
    import concourse.bass as bass
    import concourse.bacc as bacc
    import concourse.mybir as mybir
    import concourse.tile as tile

    FP = mybir.dt.float32
    AF = mybir.ActivationFunctionType
    ALU = mybir.AluOpType
    AX = mybir.AxisListType

    OPS = _register_custom_ops()
    RSQ_F, RSQ_NR, ZS_F, F_F, GN2_F = (
        OPS["ANT_RSQ_F"], OPS["ANT_RSQ_NR"], OPS["ANT_ZS_F"],
        OPS["ANT_F_F"], OPS["ANT_GN2_F"])


    INT32 = mybir.dt.int32

    def emit_rsqrt(pool, u_t, shape, tag, nr=True):
        """r = rsqrt(u): DVE shift/xor seed + custom quake op (+Newton)."""
        seed = pool.tile(shape, FP, tag=tag + "_sd")
        nc.vector.tensor_scalar(seed[:, :].bitcast(INT32),
                                u_t[:, :].bitcast(INT32), 1, -1,
                                ALU.logical_shift_right, ALU.bitwise_xor)
        y_t = pool.tile(shape, FP, tag=tag + "_y")
        nc.vector._custom_dve(RSQ_F, out=y_t[:, :], in0=u_t[:, :],
                              in1=seed[:, :], s0=RSQ_C1, s1=RSQ_C2)
        if not nr:
            return y_t
        r_t = pool.tile(shape, FP, tag=tag + "_r")
        nc.vector._custom_dve(RSQ_NR, out=r_t[:, :], in0=u_t[:, :],
                              in1=y_t[:, :], s0=1.5, s1=0.5)
        return r_t

    nc = bacc.Bacc()
    x_d = nc.dram_tensor("x_shard", (ROWS_PER_CORE, C_IN, D_DIM), FP,
                         kind="ExternalInput")
    w_d = nc.dram_tensor("w_mat", (C_IN, C_OUT), FP, kind="ExternalInput")
    id_d = nc.dram_tensor("ident", (64, 64), FP, kind="ExternalInput")
    out_d = nc.dram_tensor("out_shard", (ROWS_PER_CORE, C_OUT, D_DIM), FP,
                           kind="ExternalOutput")

    ctx = ExitStack()
    with ctx:
        tc = ctx.enter_context(tile.TileContext(nc))
        const = ctx.enter_context(tc.tile_pool(name="const", bufs=1))
        xg_p = ctx.enter_context(tc.tile_pool(name="xg", bufs=B["xg"]))
        work = ctx.enter_context(tc.tile_pool(name="work", bufs=B["work"]))
        ab_p = ctx.enter_context(tc.tile_pool(name="ab", bufs=B["ab"]))
        red_p = ctx.enter_context(tc.tile_pool(name="red", bufs=B["red"]))
        psum = ctx.enter_context(tc.tile_pool(name="ps", bufs=B["ps"],
                                              space="PSUM"))

        R = rpg         # rows per group, all at partitions 0-63
        W = 64 * R      # free elems per group tile

        # ---- constants
        w_g = const.tile([128, W], FP, tag="wg")
        for r in range(R):
            nc.sync.dma_start(w_g[0:64, 64 * r:64 * r + 64], w_d[:, :])
            nc.sync.dma_start(w_g[64:128, 64 * r:64 * r + 64], w_d[:, :])
        ident = const.tile([128, 64], FP, tag="ident")
        nc.sync.dma_start(ident[0:64, :], id_d[:, :])
        nc.sync.dma_start(ident[64:128, :], id_d[:, :])
        ones = const.tile([64, 64], FP, tag="ones")
        nc.vector.memset(ones[:, :], 1.0)
        halfpi = const.tile([128, 1], FP, tag="halfpi")
        nc.vector.memset(halfpi[:, :], HALF_PI)

        def b3(t):  # (64, W) -> (64, R, 64) view
            return t[:, :].rearrange("p (j d) -> p j d", d=64)

        def emit_load(st):
            """Load a PAIR of row-groups: A -> partitions 0-63, B -> 64-127."""
            n0a, n0b = st["n0a"], st["n0b"]
            X = xg_p.tile([128, W], FP, tag="xg")
            nc.sync.dma_start(
                X[0:64, :].rearrange("p (j d) -> p j d", d=64),
                x_d[n0a:n0a + R].rearrange("j i d -> i j d"))
            nc.sync.dma_start(
                X[64:128, :].rearrange("p (j d) -> p j d", d=64),
                x_d[n0b:n0b + R].rearrange("j i d -> i j d"))
            XT = {}
            for h, base in (("a", 0), ("b", 64)):
                tp = psum.tile([64, W], FP, tag="tp")
                for r in range(R):
                    nc.tensor.transpose(tp[:, 64 * r:64 * r + 64],
                                        X[base:base + 64, 64 * r:64 * r + 64],
                                        ident[base:base + 64, :])
                XT[h] = xg_p.tile([64, W], FP, tag="xt" + h, name="xt" + h)
                nc.scalar.copy(XT[h][:, :], tp[:, :])
            st["X"] = X
            st["XT"] = XT

        def lift(dst_up, src64, tag_unused=None):
            # SBUF->SBUF DMA moving a (64, *) tile into partitions 64-127
            nc.sync.dma_start(dst_up, src64[:, :])

        def fchain(Dc, shape, tagp):
            """factor chain on a stacked tile: returns S-factor f (same shape)."""
            q = work.tile(shape, FP, tag=tagp + "q")
            nc.scalar.activation(q[:, :], Dc[:, :], AF.Square)
            u = work.tile(shape, FP, tag=tagp + "u")
            nc.vector.tensor_scalar(u[:, :], q[:, :], -1.0, 1.0,
                                    ALU.mult, ALU.add)
            rr = emit_rsqrt(work, u, shape, tagp + "r", nr=False)
            zs = work.tile(shape, FP, tag=tagp + "z")
            nc.vector._custom_dve(ZS_F, out=zs[:, :], in0=Dc[:, :],
                                  in1=rr[:, :], s1=-2.0)
            th = work.tile(shape, FP, tag=tagp + "t")
            nc.scalar.activation(th[:, :], zs[:, :], AF.Arctan)
            f = work.tile(shape, FP, tag=tagp + "f")
            nc.vector._custom_dve(F_F, out=f[:, :], in0=th[:, :],
                                  in1=rr[:, :], s0=PI, s1=2.0)
            return f

        def emit_factor(st, it):
            X, XT = st["X"], st["XT"]
            if it == 0:
                Dc0 = red_p.tile([128, R], FP, tag="dc0")
                for h, base in (("a", 0), ("b", 64)):
                    dcol = psum.tile([64, R], FP, tag="mm")
                    for r in range(R):
                        nc.tensor.matmul(dcol[:, r:r + 1],
                                         XT[h][:, 64 * r:64 * r + 64],
                                         XT[h][:, 64 * r:64 * r + 1])
                    if base == 0:
                        nc.vector.tensor_scalar(Dc0[0:64, :], dcol[:, :],
                                                CLIP, -CLIP, ALU.min, ALU.max)
                    else:
                        tmp = red_p.tile([64, R], FP, tag="dc0t")
                        nc.vector.tensor_scalar(tmp[:, :], dcol[:, :],
                                                CLIP, -CLIP, ALU.min, ALU.max)
                        lift(Dc0[64:128, :], tmp)
                f0 = fchain(Dc0, [128, R], "f0")
                S = work.tile([128, W], FP, tag="sg")
                for r in range(R):
                    nc.vector.tensor_scalar(S[:, 64 * r:64 * r + 64],
                                            w_g[:, 64 * r:64 * r + 64],
                                            f0[:, r:r + 1], None, ALU.mult)
                A = ab_p.tile([128, W], FP, tag="ag")
                for (n0, sl) in ((st["n0a"], slice(0, 64)),
                                 (st["n0b"], slice(64, 128))):
                    nc.sync.dma_start(
                        A[sl, :].rearrange("p (j d) -> p j d", d=64),
                        x_d[n0:n0 + R, 0:1, :].rearrange("j o d -> o j d")
                        .broadcast_to([64, R, 64]))
                st["A"] = A
            else:
                AT = st["AT"]
                Dc = work.tile([128, W], FP, tag="dcf")
                for h, base in (("a", 0), ("b", 64)):
                    dt = psum.tile([64, W], FP, tag="mm")
                    for r in range(R):
                        nc.tensor.matmul(dt[:, 64 * r:64 * r + 64],
                                         XT[h][:, 64 * r:64 * r + 64],
                                         AT[h][:, 64 * r:64 * r + 64])
                    if base == 0:
                        nc.vector.tensor_scalar(Dc[0:64, :], dt[:, :],
                                                CLIP, -CLIP, ALU.min, ALU.max)
                    else:
                        tmp = work.tile([64, W], FP, tag="dcft")
                        nc.vector.tensor_scalar(tmp[:, :], dt[:, :],
                                                CLIP, -CLIP, ALU.min, ALU.max)
                        lift(Dc[64:128, :], tmp)
                ff = fchain(Dc, [128, W], "ff")
                S = work.tile([128, W], FP, tag="sg")
                nc.vector.tensor_tensor(S[:, :], w_g[:, :], ff[:, :], ALU.mult)
            st["S"] = S

        def b3s(t):
            return t[:, :].rearrange("p (j d) -> p j d", d=64)

        def emit_update(st, it):
            X, S, A = st["X"], st["S"], st["A"]
            gsb = work.tile([128, W], FP, tag="gsb")
            for h, base in (("a", 0), ("b", 64)):
                gp = psum.tile([64, W], FP, tag="mm")
                for r in range(R):
                    nc.tensor.matmul(gp[:, 64 * r:64 * r + 64],
                                     S[base:base + 64, 64 * r:64 * r + 64],
                                     X[base:base + 64, 64 * r:64 * r + 64])
                if base == 0:
                    nc.scalar.copy(gsb[0:64, :], gp[:, :])
                else:
                    tmp = work.tile([64, W], FP, tag="gsbt")
                    nc.scalar.copy(tmp[:, :], gp[:, :])
                    lift(gsb[64:128, :], tmp)
            prod = work.tile([128, W], FP, tag="scr1")
            nc.vector.tensor_tensor(prod[:, :], A[:, :], gsb[:, :], ALU.mult)
            coefA = red_p.tile([128, R], FP, tag="coef")
            nc.vector.tensor_reduce(coefA[:, :], b3s(prod), AX.X, ALU.add)
            g2 = work.tile([128, W], FP, tag="scr2")
            nc.scalar.activation(g2[:, :], gsb[:, :], AF.Square)
            gn2r = red_p.tile([128, R], FP, tag="gn2r")
            nc.vector.tensor_reduce(gn2r[:, :], b3s(g2), AX.X, ALU.add)
            gn2 = red_p.tile([128, R], FP, tag="gn2")
            nc.vector._custom_dve(GN2_F, out=gn2[:, :], in0=gn2r[:, :],
                                  in1=coefA[:, :], s0=1e-30)
            rg = emit_rsqrt(red_p, gn2, [128, R], "rg")
            gn = red_p.tile([128, R], FP, tag="gn")
            nc.vector.tensor_tensor(gn[:, :], gn2[:, :], rg[:, :], ALU.mult)
            cosg = red_p.tile([128, R], FP, tag="cosg")
            nc.scalar.activation(cosg[:, :], gn[:, :], AF.Sin,
                                 bias=halfpi[:, 0:1])
            s1t = red_p.tile([128, R], FP, tag="s1t")
            nc.scalar.activation(s1t[:, :], gn[:, :], AF.Sin)
            sc = red_p.tile([128, R], FP, tag="sc")
            nc.vector.tensor_tensor(sc[:, :], s1t[:, :], rg[:, :], ALU.mult)
            t9 = red_p.tile([128, R], FP, tag="t9")
            nc.vector.tensor_tensor(t9[:, :], sc[:, :], coefA[:, :], ALU.mult)
            alpha = red_p.tile([128, R], FP, tag="alpha")
            nc.vector.tensor_tensor(alpha[:, :], cosg[:, :], t9[:, :],
                                    ALU.subtract)
            sc_b = sc[:, :].rearrange("p (j o) -> p j o", o=1)\
                .broadcast_to([128, R, 64])
            al_b = alpha[:, :].rearrange("p (j o) -> p j o", o=1)\
                .broadcast_to([128, R, 64])
            t2 = work.tile([128, W], FP, tag="scr1")
            nc.vector.tensor_tensor(b3s(t2), b3s(gsb), sc_b, ALU.mult)
            t1 = work.tile([128, W], FP, tag="scr2")
            nc.vector.tensor_tensor(b3s(t1), b3s(A), al_b, ALU.mult)
            An = ab_p.tile([128, W], FP, tag="ag")
            nc.vector.tensor_tensor(An[:, :], t1[:, :], t2[:, :], ALU.add)
            st["A"] = An
            if it < 2:
                AT = {}
                for h, base in (("a", 0), ("b", 64)):
                    tpa = psum.tile([64, W], FP, tag="tp")
                    for r in range(R):
                        nc.tensor.transpose(
                            tpa[:, 64 * r:64 * r + 64],
                            An[base:base + 64, 64 * r:64 * r + 64],
                            ident[base:base + 64, :])
                    AT[h] = ab_p.tile([64, W], FP, tag="at" + h, name="at" + h)
                    nc.scalar.copy(AT[h][:, :], tpa[:, :])
                st["AT"] = AT
            else:
                nc.sync.dma_start(
                    out_d[st["n0a"]:st["n0a"] + R].rearrange("j o d -> o j d"),
                    An[0:64, :].rearrange("p (j d) -> p j d", d=64))
                nc.sync.dma_start(
                    out_d[st["n0b"]:st["n0b"] + R].rearrange("j o d -> o j d"),
                    An[64:128, :].rearrange("p (j d) -> p j d", d=64))

        n_pairs = n_groups // 2
        for rep in range(repeat):
            for p0 in range(0, n_pairs, INTERLEAVE):
                sts = []
                for p in range(p0, min(p0 + INTERLEAVE, n_pairs)):
                    sts.append({"n0a": rpg * (2 * p), "n0b": rpg * (2 * p + 1)})
                for st in sts:
                    emit_load(st)
                for it in range(3):
                    for st in sts:
                        emit_factor(st, it)
                    for st in sts:
                        emit_update(st, it)
    nc.compile()
    return nc


# ---------------------------------------------------------------------------
# host entry point

def _get_program():
    if "nc" not in _COMPILED:
        _COMPILED["nc"] = build_program()
    return _COMPILED["nc"]


def _ensure_trace_hook():
    """Register the NTFF profile hook (the image's antenv lacks axon_hooks)."""
    try:
        from antenv.axon_hooks import get_axon_ntff_profile_hook
        return get_axon_ntff_profile_hook() is not None
    except ImportError:
        pass
    try:
        import sys, types
        import antenv
        from trn_agent_boot.trn_boot import _ntff_profile_via_ctypes
        mod = types.ModuleType("antenv.axon_hooks")
        _h = {}
        mod.set_axon_ntff_profile_hook = lambda h: _h.__setitem__("h", h)
        mod.get_axon_ntff_profile_hook = lambda: _h.get("h")
        sys.modules["antenv.axon_hooks"] = mod
        antenv.axon_hooks = mod
        mod.set_axon_ntff_profile_hook(
            _ntff_profile_via_ctypes("/opt/axon/libaxon_pjrt.so"))
        return True
    except Exception:
        return False


def kernel(x, w_raw, _trace=False):
    from concourse.bass_utils import run_bass_kernel_spmd
    if _trace:
        _trace = _ensure_trace_hook()

    x = np.ascontiguousarray(np.asarray(x, f32))
    w_raw = np.asarray(w_raw, f32)
    B, L, C_in, d = x.shape
    N = B * L
    w = np.exp((w_raw - f32(np.log(C_in))).astype(f32)).astype(f32)
    w = (w / w.sum(axis=0, keepdims=True)).astype(f32)
    ident = np.eye(64, dtype=f32)

    xr = x.reshape(N, C_in, d)
    nc = _get_program()
    in_maps = []
    for k in range(N_CORES):
        in_maps.append({
            "x_shard": xr[k * ROWS_PER_CORE:(k + 1) * ROWS_PER_CORE],
            "w_mat": w,
            "ident": ident,
        })
    res = run_bass_kernel_spmd(nc, in_maps, core_ids=list(range(N_CORES)),
                               trace=_trace)
    out = np.concatenate([res.results[k]["out_shard"] for k in range(N_CORES)],
                         axis=0)
    if _trace:
        kernel.last_exec_time_ns = res.exec_time_ns
        kernel.last_results = res
    return out.reshape(B, L, C_OUT, d)



# revision 3
# speedup vs baseline: 1.8334x; 1.8334x over previous
"""Trainium2 Bass kernel v3 for nn_MfdFC. See kernel_v2 docstring for math.

v3 over v2:
- host pre-transposes x into per-(block,half) [i, j, d] layout and w into the
  replicated [128, 1024] SBUF image -> all input DMAs are contiguous; the
  output is written in SBUF-natural [o, j, d] order and re-transposed on host.
- the per-block [128,16] "smalls" pipelines (iter-0 f-chain, update-phase
  cos/sin/rsqrt chain) run once per interleave-PAIR on [128,32] tiles, and
  the coefA/|G|^2 reductions of a pair are fused into ONE 4096-wide reduce.
- optional GPSIMD offload for selected elementwise passes (t2, prod).
"""
import math
import numpy as np

f32 = np.float32

C_IN = 64
C_OUT = 64
D_DIM = 64
ROWS_PER_CORE = 128
N_CORES = 8
R = 16
NBLK = 4
W = 64 * R
RSQ_C1 = 1.7584694439735017e-30
RSQ_C2 = -2.755803843779718e-20
HALF_PI = float(f32(math.pi / 2.0))
EPS_U = float(f32(2.0 ** -22))

_COMPILED = {}

def _register_custom_ops():
    import concourse.dve_ops as dve_ops
    from concourse.dve_ops import DveOp
    from concourse.dve_spec import (
        Spec, Src0, Src1, C0, C1, lower, maxx, _has_src1 as has_src1,
    )
    from concourse.dve_uop import DveOpSpec
    from concourse.dve_table_gen import dve_ver_for

    if "ANT_RSQ_F" in dve_ops._SUB_OPCODE_FOR_NAME:
        return {n: op for n, op in ((o.name, o) for o in dve_ops.OPS)
                if n.startswith("ANT_")}

    def _ref_rsq_f(in0, in1, s0, s1, imm2):
        u = np.asarray(in0, f32)
        nt = np.asarray(in1, f32)
        m1 = (nt * f32(s0)).astype(f32)
        m2 = (m1 * nt).astype(f32)
        m3 = (m2 * f32(s0)).astype(f32)
        t = (m3 * u).astype(f32)
        return ((t + f32(s1)) * nt).astype(f32)

    _m1 = Src1 * C0
    _m3 = (_m1 * Src1) * C0
    RSQ_F = DveOp("ANT_RSQ_F",
                  Spec(body=((_m3 * Src0) + C1) * Src1, reference=_ref_rsq_f),
                  subdim=False, uops_sha={})

    def _ref_rsq_nr(in0, in1, s0, s1, imm2):
        u = np.asarray(in0, f32); y = np.asarray(in1, f32)
        a = (u * y).astype(f32)
        b = (a * y).astype(f32)
        return ((f32(s0) - (b * f32(s1)).astype(f32)) * y).astype(f32)

    RSQ_NR = DveOp("ANT_RSQ_NR",
                   Spec(body=(C0 - ((Src0 * Src1) * Src1) * C1) * Src1,
                        reference=_ref_rsq_nr),
                   subdim=False, uops_sha={})

    def _ref_gn2(in0, in1, s0, s1, imm2):
        raw = np.asarray(in0, f32); c = np.asarray(in1, f32)
        return np.maximum((raw - (c * c).astype(f32)).astype(f32), f32(s0))

    GN2_F = DveOp("ANT_GN2_F",
                  Spec(body=maxx(Src0 - Src1 * Src1, C0), reference=_ref_gn2),
                  subdim=False, uops_sha={})

    ops = [RSQ_F, RSQ_NR, GN2_F]
    base = dve_ops._CUSTOM_DVE_ROW_BASE + len(dve_ops.OPS)
    for i, op in enumerate(ops):
        dve_ops._SUB_OPCODE_FOR_NAME[op.name] = base + i
    for trn in ("TRN2",):
        ver = dve_ver_for(trn)
        for op in ops:
            uops = lower(op.spec, ver=ver)
            s = DveOpSpec(name=op.name, opcode=dve_ops.get_dve_sub_opcode(op.name),
                          uops=uops, rd1_en=has_src1(op.spec))
            op.uops_sha[ver] = s.sha(ver)
    dve_ops.OPS.extend(ops)
    dve_ops.CUSTOM_DVE_SPECS.update({op.name: op.spec for op in ops})
    return {op.name: op for op in ops}



def _ensure_trace_hook():
    try:
        from antenv.axon_hooks import get_axon_ntff_profile_hook
        return get_axon_ntff_profile_hook() is not None
    except ImportError:
        pass
    try:
        import sys, types
        import antenv
        from trn_agent_boot.trn_boot import _ntff_profile_via_ctypes
        mod = types.ModuleType("antenv.axon_hooks")
        _h = {}
        mod.set_axon_ntff_profile_hook = lambda h: _h.__setitem__("h", h)
        mod.get_axon_ntff_profile_hook = lambda: _h.get("h")
        sys.modules["antenv.axon_hooks"] = mod
        antenv.axon_hooks = mod
        mod.set_axon_ntff_profile_hook(
            _ntff_profile_via_ctypes("/opt/axon/libaxon_pjrt.so"))
        return True
    except Exception:
        return False





def build_program(INTERLEAVE=2, gps=(), redsplit=False, wbufs=2):
    from contextlib import ExitStack
    import concourse.bacc as bacc
    import concourse.mybir as mybir
    import concourse.tile as tile

    gps = frozenset(gps)
    FP = mybir.dt.float32
    BF = mybir.dt.bfloat16
    I32 = mybir.dt.int32
    AF = mybir.ActivationFunctionType
    ALU = mybir.AluOpType
    AX = mybir.AxisListType

    OPS = _register_custom_ops()
    RSQ_F, RSQ_NR, GN2_F = OPS["ANT_RSQ_F"], OPS["ANT_RSQ_NR"], OPS["ANT_GN2_F"]

    nc = bacc.Bacc()
    # x pre-transposed on host: [block, half, i, j, d]
    x_d = nc.dram_tensor("xp", (NBLK, 2, C_IN, R, D_DIM), BF,
                         kind="ExternalInput")
    # x0 rows: [block, half, j, d]
    x0_d = nc.dram_tensor("x0p", (NBLK, 2, R, D_DIM), BF, kind="ExternalInput")
    w_d = nc.dram_tensor("w_rep", (128, W), BF, kind="ExternalInput")
    id_d = nc.dram_tensor("ident2", (128, 64), BF, kind="ExternalInput")
    # output in SBUF-natural order: [block, half, o, j, d]
    out_d = nc.dram_tensor("out_p", (NBLK, 2, C_OUT, R, D_DIM), FP,
                           kind="ExternalOutput")

    ctx = ExitStack()
    with ctx:
        tc = ctx.enter_context(tile.TileContext(nc))
        const = ctx.enter_context(tc.tile_pool(name="const", bufs=1))
        xg_p = ctx.enter_context(tc.tile_pool(name="xg", bufs=max(2, INTERLEAVE)))
        work = ctx.enter_context(tc.tile_pool(name="work", bufs=wbufs))
        ab_p = ctx.enter_context(tc.tile_pool(name="ab", bufs=max(2, INTERLEAVE)))
        red_p = ctx.enter_context(tc.tile_pool(name="red", bufs=2))
        ps_f = ctx.enter_context(tc.tile_pool(name="psf", bufs=2, space="PSUM"))
        ps_t = ctx.enter_context(tc.tile_pool(name="pst", bufs=2, space="PSUM"))
        ps_s = ctx.enter_context(tc.tile_pool(name="pss", bufs=1, space="PSUM"))

        def eng(name):
            return nc.gpsimd if name in gps else nc.vector

        # ---- constants (all contiguous DMAs)
        w_g = const.tile([128, W], BF, tag="wg")
        nc.sync.dma_start(w_g[:, :], w_d[:, :])
        ident = const.tile([128, 64], BF, tag="ident")
        nc.sync.dma_start(ident[:, :], id_d[:, :])
        halfpi = const.tile([128, 1], FP, tag="halfpi")
        nc.vector.memset(halfpi[:, :], HALF_PI)

        def jbh(t, h, j):
            return t[64 * h:64 * h + 64, 64 * j:64 * j + 64]

        def b3(t):
            return t[:, :].rearrange("p (j d) -> p j d", d=64)

        def bcR(small_ap):      # [128, R] ap -> broadcast (p, j, 64)
            return small_ap.rearrange("p (j o) -> p j o", o=1) \
                .broadcast_to([128, R, 64])

        def emit_load(st):
            b = st["b"]
            X = xg_p.tile([128, W], BF, tag="xg")
            for h in (0, 1):
                nc.sync.dma_start(b3(X[64 * h:64 * h + 64, :]), x_d[b, h])
            A0 = ab_p.tile([128, W], BF, tag="ag")
            for h in (0, 1):
                nc.sync.dma_start(
                    A0[64 * h:64 * h + 64, :].rearrange("p (j d) -> p j d", d=64),
                    x0_d[b:b + 1, h].rearrange("b j d -> b j d")
                    .broadcast_to([64, R, 64]))
            tp = ps_t.tile([128, W], BF, tag="tp")
            for h in (0, 1):
                for r in range(R):
                    nc.tensor.transpose(jbh(tp, h, r), jbh(X, h, r),
                                        ident[64 * h:64 * h + 64, :])
            XT = xg_p.tile([128, W], BF, tag="xt")
            nc.scalar.copy(XT[:, :], tp[:, :])
            st["X"], st["XT"], st["A"] = X, XT, A0

        def quake(pool, src_ap, shape, tagp, out_dt, nr=False):
            seed = pool.tile(shape, FP, tag=tagp + "sd")
            nc.vector.tensor_scalar(seed[:, :].bitcast(I32),
                                    src_ap.bitcast(I32), 1, -1,
                                    ALU.logical_shift_right, ALU.bitwise_xor)
            rr = pool.tile(shape, out_dt, tag=tagp + "rr")
            nc.vector._custom_dve(RSQ_F, out=rr[:, :], in0=src_ap,
                                  in1=seed[:, :], s0=RSQ_C1, s1=RSQ_C2)
            if not nr:
                return rr
            rr2 = pool.tile(shape, out_dt, tag=tagp + "r2")
            nc.vector._custom_dve(RSQ_NR, out=rr2[:, :], in0=src_ap,
                                  in1=rr[:, :], s0=1.5, s1=0.5)
            return rr2

        # ---------- iteration 0: D0 for the whole pair in one [128, 2*R] tile
        def emit_d0(sts):
            npair = len(sts)
            psD0 = ps_s.tile([128, npair * R], FP, tag="d0")
            for idx, st in enumerate(sts):
                XT = st["XT"]
                for h in (0, 1):
                    for r in range(R):
                        nc.tensor.matmul(
                            psD0[64 * h:64 * h + 64, R * idx + r:R * idx + r + 1],
                            jbh(XT, h, r),
                            XT[64 * h:64 * h + 64, 64 * r:64 * r + 1])
            shape = [128, npair * R]
            q0 = red_p.tile(shape, FP, tag="f0q")
            nc.scalar.activation(q0[:, :], psD0[:, :], AF.Square)
            u0 = red_p.tile(shape, FP, tag="f0u")
            nc.vector.tensor_scalar(u0[:, :], q0[:, :], -1.0, 1.0 + EPS_U,
                                    ALU.mult, ALU.add)
            rr0 = quake(red_p, u0[:, :], shape, "f0", FP)
            zs0 = red_p.tile(shape, FP, tag="f0z")
            nc.vector.tensor_tensor(zs0[:, :], psD0[:, :], rr0[:, :], ALU.mult)
            th0 = red_p.tile(shape, FP, tag="f0t")
            nc.scalar.activation(th0[:, :], zs0[:, :], AF.Arctan, scale=-1.0)
            f0 = red_p.tile(shape, BF, tag="f0v")
            nc.vector.scalar_tensor_tensor(f0[:, :], th0[:, :], HALF_PI,
                                           rr0[:, :], ALU.add, ALU.mult)
            for idx, st in enumerate(sts):
                st["f0"] = f0[:, R * idx:R * idx + R]

        def emit_factor(st, it):
            X, XT = st["X"], st["XT"]
            if it == 0:
                Xf = work.tile([128, W], BF, tag="xf")
                nc.vector.tensor_tensor(b3(Xf), b3(X), bcR(st["f0"]), ALU.mult)
                st["Xf"] = Xf
                return
            AT = st["AT"]
            psD = ps_f.tile([128, W], FP, tag="mmf")
            for h in (0, 1):
                for r in range(R):
                    nc.tensor.matmul(jbh(psD, h, r), jbh(XT, h, r),
                                     jbh(AT, h, r))
            Dd = work.tile([128, W], BF, tag="dd")
            nc.scalar.copy(Dd[:, :], psD[:, :])
            q = work.tile([128, W], FP, tag="ffq")
            nc.scalar.activation(q[:, :], psD[:, :], AF.Square)
            u = work.tile([128, W], FP, tag="ffu")
            eng("u").tensor_scalar(u[:, :], q[:, :], -1.0, 1.0 + EPS_U,
                                   ALU.mult, ALU.add)
            rr = quake(work, u[:, :], [128, W], "ff", BF)
            zs = work.tile([128, W], BF, tag="zs")
            nc.vector.tensor_tensor(zs[:, :], Dd[:, :], rr[:, :], ALU.mult)
            th = work.tile([128, W], BF, tag="th")
            nc.scalar.activation(th[:, :], zs[:, :], AF.Arctan, scale=-1.0)
            thp = work.tile([128, W], BF, tag="thp")
            nc.vector.tensor_scalar(thp[:, :], th[:, :], HALF_PI, None,
                                    ALU.add)
            f = work.tile([128, W], BF, tag="fv")
            nc.vector.tensor_tensor(f[:, :], thp[:, :], rr[:, :], ALU.mult)
            S = work.tile([128, W], BF, tag="sg")
            nc.vector.tensor_tensor(S[:, :], w_g[:, :], f[:, :], ALU.mult)
            st["S"] = S

        def emit_gmm(st, it):
            X = st["X"]
            psG = ps_f.tile([128, W], FP, tag="mmf")
            if it == 0:
                Xf = st["Xf"]
                for h in (0, 1):
                    for c in (0, 512):
                        nc.tensor.matmul(psG[64 * h:64 * h + 64, c:c + 512],
                                         w_g[64 * h:64 * h + 64, 0:64],
                                         Xf[64 * h:64 * h + 64, c:c + 512])
            else:
                S = st["S"]
                for h in (0, 1):
                    for r in range(R):
                        nc.tensor.matmul(jbh(psG, h, r), jbh(S, h, r),
                                         jbh(X, h, r))
            Gd = work.tile([128, W], BF, tag="gd")
            nc.scalar.copy(Gd[:, :], psG[:, :])
            st["Gd"], st["psG"] = Gd, psG

        def emit_update(sts, it):
            # per-pair combined reduce over [prod_A, g2_A, prod_B, g2_B]
            npair = len(sts)
            pg = work.tile([128, 2 * W * npair], BF, tag="pg")
            for idx, st in enumerate(sts):
                off = 2 * W * idx
                eng("prod").tensor_tensor(pg[:, off:off + W], st["A"][:, :],
                                          st["Gd"][:, :], ALU.mult)
                nc.scalar.activation(pg[:, off + W:off + 2 * W],
                                     st["psG"][:, :], AF.Square)
            nred = 2 * R * npair
            red = red_p.tile([128, nred], FP, tag="red")
            rview = pg[:, :].rearrange("p (s d) -> p s d", d=64)
            if redsplit and npair == 2:
                nc.vector.tensor_reduce(
                    red[:, 0:2 * R].rearrange("p (s j) -> p s j", j=R),
                    rview[:, 0:32], AX.X, ALU.add)
                nc.gpsimd.tensor_reduce(
                    red[:, 2 * R:].rearrange("p (s j) -> p s j", j=R),
                    rview[:, 32:64], AX.X, ALU.add)
            else:
                nc.vector.tensor_reduce(red[:, :], rview, AX.X, ALU.add)
            # red cols: [idx][kind][j]: coefA at kind 0, gnr at kind 1
            rv = red[:, :].rearrange("p (i k j) -> p i k j", k=2, j=R)
            shape = [128, R * npair]
            coefA = red[:, :].rearrange("p (i k j) -> p (i k) j", k=2, j=R)
            # strided views
            cview = rv[:, :, 0, :]          # [128, npair, R]
            gview = rv[:, :, 1, :]
            gn2 = red_p.tile(shape, FP, tag="gn2")
            g3 = gn2[:, :].rearrange("p (i j) -> p i j", j=R)
            nc.vector._custom_dve(GN2_F, out=g3, in0=gview, in1=cview,
                                  s0=1e-30)
            rg = quake(red_p, gn2[:, :], shape, "rg", FP, nr=True)
            gn = red_p.tile(shape, FP, tag="gn")
            nc.vector.tensor_tensor(gn[:, :], gn2[:, :], rg[:, :], ALU.mult)
            cosg = red_p.tile(shape, FP, tag="cosg")
            nc.scalar.activation(cosg[:, :], gn[:, :], AF.Sin,
                                 bias=halfpi[:, 0:1])
            s1t = red_p.tile(shape, FP, tag="s1t")
            nc.scalar.activation(s1t[:, :], gn[:, :], AF.Sin)
            sc = red_p.tile(shape, FP, tag="sc")
            nc.vector.tensor_tensor(sc[:, :], s1t[:, :], rg[:, :], ALU.mult)
            t9 = red_p.tile(shape, FP, tag="t9")
            nc.vector.scalar_tensor_tensor(
                t9[:, :].rearrange("p (i j) -> p i j", j=R), sc[:, :]
                .rearrange("p (i j) -> p i j", j=R), -1.0, cview,
                ALU.mult, ALU.mult)
            alpha = red_p.tile(shape, BF, tag="alpha")
            nc.vector.tensor_tensor(alpha[:, :], cosg[:, :], t9[:, :], ALU.add)
            scb = red_p.tile(shape, BF, tag="scb")
            nc.vector.tensor_copy(scb[:, :], sc[:, :])
            last = it == 2
            for idx, st in enumerate(sts):
                A, Gd = st["A"], st["Gd"]
                t1 = work.tile([128, W], BF, tag="scr1")
                eng("t1").tensor_tensor(b3(t1), b3(A),
                                        bcR(alpha[:, R * idx:R * idx + R]),
                                        ALU.mult)
                t2 = work.tile([128, W], BF, tag="scr2")
                eng("t2").tensor_tensor(b3(t2), b3(Gd),
                                        bcR(scb[:, R * idx:R * idx + R]),
                                        ALU.mult)
                An = ab_p.tile([128, W], FP if last else BF,
                               tag="agf" if last else "ag")
                nc.vector.tensor_tensor(An[:, :], t1[:, :], t2[:, :], ALU.add)
                st["A"] = An
                if not last:
                    tp = ps_t.tile([128, W], BF, tag="tp")
                    for h in (0, 1):
                        for r in range(R):
                            nc.tensor.transpose(jbh(tp, h, r), jbh(An, h, r),
                                                ident[64 * h:64 * h + 64, :])
                    AT = ab_p.tile([128, W], BF, tag="at")
                    nc.scalar.copy(AT[:, :], tp[:, :])
                    st["AT"] = AT
                else:
                    for h in (0, 1):
                        nc.sync.dma_start(
                            out_d[st["b"], h],
                            b3(An[64 * h:64 * h + 64, :]))

        for b0 in range(0, NBLK, INTERLEAVE):
            sts = [{"b": b} for b in range(b0, min(b0 + INTERLEAVE, NBLK))]
            for st in sts:
                emit_load(st)
            emit_d0(sts)
            for it in range(3):
                for st in sts:
                    emit_factor(st, it)
                for st in sts:
                    emit_gmm(st, it)
                emit_update(sts, it)
    nc.compile()
    return nc


def _get_program(**kw):
    key = tuple(sorted((k, tuple(v) if isinstance(v, (list, tuple, set, frozenset))
                        else v) for k, v in kw.items()))
    if key not in _COMPILED:
        _COMPILED[key] = build_program(**kw)
    return _COMPILED[key]


def kernel(x, w_raw, _trace=False, **bkw):
    import ml_dtypes
    from concourse.bass_utils import run_bass_kernel_spmd
    if _trace:
        _trace = _ensure_trace_hook()

    bf16 = ml_dtypes.bfloat16
    x = np.asarray(x, f32)
    w_raw = np.asarray(w_raw, f32)
    B, L, C_in, d = x.shape
    N = B * L
    w = np.exp((w_raw - f32(np.log(C_in))).astype(f32)).astype(f32)
    w = (w / w.sum(axis=0, keepdims=True)).astype(f32)

    xr = x.reshape(N, C_in, d)
    # per core: [NBLK, 2, R, i, d] -> transpose to [NBLK, 2, i, R, d]
    xcore = xr.reshape(N_CORES, NBLK, 2, R, C_in, d)
    xp = np.ascontiguousarray(xcore.transpose(0, 1, 2, 4, 3, 5)).astype(bf16)
    x0p = np.ascontiguousarray(xcore[:, :, :, :, 0, :]).astype(bf16)
    w_rep = np.ascontiguousarray(
        np.broadcast_to(w.T.reshape(1, 64, 1, 64), (2, 64, R, 64))
        .transpose(0, 3, 2, 1).reshape(128, W)).astype(bf16)
    # w_rep[p, (j, o)]: lower/upper halves identical, = w[i=p%64, o]
    w_rep = np.ascontiguousarray(
        np.tile(np.repeat(w[None, :, :], 1, axis=0), (2, 1, 1))  # (2,64,64)
        .reshape(2, 64, 1, 64).repeat(R, axis=2).reshape(2 * 64, R * 64)
        ).astype(bf16)
    ident2 = np.tile(np.eye(64, dtype=bf16), (2, 1))

    nc = _get_program(**bkw)
    in_maps = []
    for k in range(N_CORES):
        in_maps.append({
            "xp": xp[k],
            "x0p": x0p[k],
            "w_rep": w_rep,
            "ident2": ident2,
        })
    res = run_bass_kernel_spmd(nc, in_maps, core_ids=list(range(N_CORES)),
                               trace=_trace)
    # out_p: [NBLK, 2, o, j, d] per core -> rows
    outs = []
    for k in range(N_CORES):
        op = res.results[k]["out_p"]          # (NBLK, 2, 64, R, 64)
        outs.append(np.ascontiguousarray(op.transpose(0, 1, 3, 2, 4))
                    .reshape(ROWS_PER_CORE, C_OUT, d))
    out = np.concatenate(outs, axis=0)
    if _trace:
        kernel.last_exec_time_ns = res.exec_time_ns
        kernel.last_results = res
    return out.reshape(B, L, C_OUT, d).astype(f32)


# revision 4
# speedup vs baseline: 2.2304x; 1.2165x over previous
"""Trainium2 Bass kernel v3 for nn_MfdFC. See kernel_v2 docstring for math.

v3 over v2:
- host pre-transposes x into per-(block,half) [i, j, d] layout and w into the
  replicated [128, 1024] SBUF image -> all input DMAs are contiguous; the
  output is written in SBUF-natural [o, j, d] order and re-transposed on host.
- the per-block [128,16] "smalls" pipelines (iter-0 f-chain, update-phase
  cos/sin/rsqrt chain) run once per interleave-PAIR on [128,32] tiles, and
  the coefA/|G|^2 reductions of a pair are fused into ONE 4096-wide reduce.
- optional GPSIMD offload for selected elementwise passes (t2, prod).
"""
import math
import numpy as np

f32 = np.float32

C_IN = 64
C_OUT = 64
D_DIM = 64
ROWS_PER_CORE = 128
N_CORES = 8
R = 16
NBLK = 4
W = 64 * R
RSQ_C1 = 1.7584694439735017e-30
RSQ_C2 = -2.755803843779718e-20
HALF_PI = float(f32(math.pi / 2.0))
EPS_U = float(f32(2.0 ** -22))

_COMPILED = {}

def _register_custom_ops():
    import concourse.dve_ops as dve_ops
    from concourse.dve_ops import DveOp
    from concourse.dve_spec import (
        Spec, Src0, Src1, C0, C1, lower, maxx, _has_src1 as has_src1,
    )
    from concourse.dve_uop import DveOpSpec
    from concourse.dve_table_gen import dve_ver_for

    if "ANT_RSQ_F" in dve_ops._SUB_OPCODE_FOR_NAME:
        return {n: op for n, op in ((o.name, o) for o in dve_ops.OPS)
                if n.startswith("ANT_")}

    def _ref_rsq_f(in0, in1, s0, s1, imm2):
        u = np.asarray(in0, f32)
        nt = np.asarray(in1, f32)
        m1 = (nt * f32(s0)).astype(f32)
        m2 = (m1 * nt).astype(f32)
        m3 = (m2 * f32(s0)).astype(f32)
        t = (m3 * u).astype(f32)
        return ((t + f32(s1)) * nt).astype(f32)

    _m1 = Src1 * C0
    _m3 = (_m1 * Src1) * C0
    RSQ_F = DveOp("ANT_RSQ_F",
                  Spec(body=((_m3 * Src0) + C1) * Src1, reference=_ref_rsq_f),
                  subdim=False, uops_sha={})

    def _ref_rsq_nr(in0, in1, s0, s1, imm2):
        u = np.asarray(in0, f32); y = np.asarray(in1, f32)
        a = (u * y).astype(f32)
        b = (a * y).astype(f32)
        return ((f32(s0) - (b * f32(s1)).astype(f32)) * y).astype(f32)

    RSQ_NR = DveOp("ANT_RSQ_NR",
                   Spec(body=(C0 - ((Src0 * Src1) * Src1) * C1) * Src1,
                        reference=_ref_rsq_nr),
                   subdim=False, uops_sha={})

    def _ref_gn2(in0, in1, s0, s1, imm2):
        raw = np.asarray(in0, f32); c = np.asarray(in1, f32)
        return np.maximum((raw - (c * c).astype(f32)).astype(f32), f32(s0))

    GN2_F = DveOp("ANT_GN2_F",
                  Spec(body=maxx(Src0 - Src1 * Src1, C0), reference=_ref_gn2),
                  subdim=False, uops_sha={})

    ops = [RSQ_F, RSQ_NR, GN2_F]
    base = dve_ops._CUSTOM_DVE_ROW_BASE + len(dve_ops.OPS)
    for i, op in enumerate(ops):
        dve_ops._SUB_OPCODE_FOR_NAME[op.name] = base + i
    for trn in ("TRN2",):
        ver = dve_ver_for(trn)
        for op in ops:
            uops = lower(op.spec, ver=ver)
            s = DveOpSpec(name=op.name, opcode=dve_ops.get_dve_sub_opcode(op.name),
                          uops=uops, rd1_en=has_src1(op.spec))
            op.uops_sha[ver] = s.sha(ver)
    dve_ops.OPS.extend(ops)
    dve_ops.CUSTOM_DVE_SPECS.update({op.name: op.spec for op in ops})
    return {op.name: op for op in ops}



def _ensure_trace_hook():
    try:
        from antenv.axon_hooks import get_axon_ntff_profile_hook
        return get_axon_ntff_profile_hook() is not None
    except ImportError:
        pass
    try:
        import sys, types
        import antenv
        from trn_agent_boot.trn_boot import _ntff_profile_via_ctypes
        mod = types.ModuleType("antenv.axon_hooks")
        _h = {}
        mod.set_axon_ntff_profile_hook = lambda h: _h.__setitem__("h", h)
        mod.get_axon_ntff_profile_hook = lambda: _h.get("h")
        sys.modules["antenv.axon_hooks"] = mod
        antenv.axon_hooks = mod
        mod.set_axon_ntff_profile_hook(
            _ntff_profile_via_ctypes("/opt/axon/libaxon_pjrt.so"))
        return True
    except Exception:
        return False





def build_program(INTERLEAVE=4, gps=(), redsplit=False, wbufs=2):
    from contextlib import ExitStack
    import concourse.bacc as bacc
    import concourse.mybir as mybir
    import concourse.tile as tile

    gps = frozenset(gps)
    FP = mybir.dt.float32
    BF = mybir.dt.bfloat16
    I32 = mybir.dt.int32
    AF = mybir.ActivationFunctionType
    ALU = mybir.AluOpType
    AX = mybir.AxisListType

    OPS = _register_custom_ops()
    RSQ_F, RSQ_NR, GN2_F = OPS["ANT_RSQ_F"], OPS["ANT_RSQ_NR"], OPS["ANT_GN2_F"]

    nc = bacc.Bacc()
    # x pre-transposed on host: [block, half, i, j, d]
    x_d = nc.dram_tensor("xp", (NBLK, 2, C_IN, R, D_DIM), BF,
                         kind="ExternalInput")
    # x0 rows: [block, half, j, d]
    x0_d = nc.dram_tensor("x0p", (NBLK, 2, R, D_DIM), BF, kind="ExternalInput")
    w_d = nc.dram_tensor("w_rep", (128, W), BF, kind="ExternalInput")
    id_d = nc.dram_tensor("ident2", (128, 64), BF, kind="ExternalInput")
    # output in SBUF-natural order: [block, half, o, j, d]
    out_d = nc.dram_tensor("out_p", (NBLK, 2, C_OUT, R, D_DIM), FP,
                           kind="ExternalOutput")

    ctx = ExitStack()
    with ctx:
        tc = ctx.enter_context(tile.TileContext(nc))
        const = ctx.enter_context(tc.tile_pool(name="const", bufs=1))
        xg_p = ctx.enter_context(tc.tile_pool(name="xg", bufs=NBLK))
        work = ctx.enter_context(tc.tile_pool(name="work", bufs=wbufs))
        deep = ctx.enter_context(tc.tile_pool(name="deep", bufs=max(2, INTERLEAVE)))
        ab_p = ctx.enter_context(tc.tile_pool(name="ab", bufs=NBLK))
        red_p = ctx.enter_context(tc.tile_pool(name="red", bufs=max(2, INTERLEAVE)))
        ps_f = ctx.enter_context(tc.tile_pool(name="psf", bufs=2, space="PSUM"))
        ps_t = ctx.enter_context(tc.tile_pool(name="pst", bufs=2, space="PSUM"))
        ps_s = ctx.enter_context(tc.tile_pool(name="pss", bufs=2, space="PSUM"))

        def eng(name):
            return nc.gpsimd if name in gps else nc.vector

        # ---- constants (all contiguous DMAs)
        w_g = const.tile([128, W], BF, tag="wg")
        nc.sync.dma_start(w_g[:, :], w_d[:, :])
        ident = const.tile([128, 64], BF, tag="ident")
        nc.sync.dma_start(ident[:, :], id_d[:, :])
        halfpi = const.tile([128, 1], FP, tag="halfpi")
        nc.vector.memset(halfpi[:, :], HALF_PI)

        def jbh(t, h, j):
            return t[64 * h:64 * h + 64, 64 * j:64 * j + 64]

        def b3(t):
            return t[:, :].rearrange("p (j d) -> p j d", d=64)

        def bcR(small_ap):      # [128, R] ap -> broadcast (p, j, 64)
            return small_ap.rearrange("p (j o) -> p j o", o=1) \
                .broadcast_to([128, R, 64])

        def emit_load(st):
            b = st["b"]
            X = xg_p.tile([128, W], BF, tag="xg")
            for h in (0, 1):
                nc.sync.dma_start(b3(X[64 * h:64 * h + 64, :]), x_d[b, h])
            A0 = ab_p.tile([128, W], BF, tag="a0")
            for h in (0, 1):
                nc.sync.dma_start(
                    A0[64 * h:64 * h + 64, :].rearrange("p (j d) -> p j d", d=64),
                    x0_d[b:b + 1, h].rearrange("b j d -> b j d")
                    .broadcast_to([64, R, 64]))
            tp = ps_t.tile([128, W], BF, tag="tp")
            for h in (0, 1):
                for r in range(R):
                    nc.tensor.transpose(jbh(tp, h, r), jbh(X, h, r),
                                        ident[64 * h:64 * h + 64, :])
            XT = xg_p.tile([128, W], BF, tag="xt")
            nc.scalar.copy(XT[:, :], tp[:, :])
            st["X"], st["XT"], st["A"] = X, XT, A0

        def quake(pool, src_ap, shape, tagp, out_dt, nr=False):
            seed = pool.tile(shape, FP, tag=tagp + "sd")
            nc.vector.tensor_scalar(seed[:, :].bitcast(I32),
                                    src_ap.bitcast(I32), 1, -1,
                                    ALU.logical_shift_right, ALU.bitwise_xor)
            rr = pool.tile(shape, out_dt, tag=tagp + "rr")
            nc.vector._custom_dve(RSQ_F, out=rr[:, :], in0=src_ap,
                                  in1=seed[:, :], s0=RSQ_C1, s1=RSQ_C2)
            if not nr:
                return rr
            rr2 = pool.tile(shape, out_dt, tag=tagp + "r2")
            nc.vector._custom_dve(RSQ_NR, out=rr2[:, :], in0=src_ap,
                                  in1=rr[:, :], s0=1.5, s1=0.5)
            return rr2

        # ---------- iteration 0: per-block D0 + small f-chain
        def emit_d0(st):
            XT = st["XT"]
            psD0 = ps_s.tile([128, R], FP, tag="d0")
            for h in (0, 1):
                for r in range(R):
                    nc.tensor.matmul(
                        psD0[64 * h:64 * h + 64, r:r + 1],
                        jbh(XT, h, r),
                        XT[64 * h:64 * h + 64, 64 * r:64 * r + 1])
            shape = [128, R]
            q0 = red_p.tile(shape, FP, tag="f0q")
            nc.scalar.activation(q0[:, :], psD0[:, :], AF.Square)
            u0 = red_p.tile(shape, FP, tag="f0u")
            nc.vector.tensor_scalar(u0[:, :], q0[:, :], -1.0, 1.0 + EPS_U,
                                    ALU.mult, ALU.add)
            rr0 = quake(red_p, u0[:, :], shape, "f0", FP)
            zs0 = red_p.tile(shape, FP, tag="f0z")
            nc.vector.tensor_tensor(zs0[:, :], psD0[:, :], rr0[:, :], ALU.mult)
            th0 = red_p.tile(shape, FP, tag="f0t")
            nc.scalar.activation(th0[:, :], zs0[:, :], AF.Arctan, scale=-1.0)
            f0 = red_p.tile(shape, BF, tag="f0v")
            nc.vector.scalar_tensor_tensor(f0[:, :], th0[:, :], HALF_PI,
                                           rr0[:, :], ALU.add, ALU.mult)
            st["f0"] = f0[:, :]

        def emit_factor(st, it):
            X, XT = st["X"], st["XT"]
            if it == 0:
                Xf = deep.tile([128, W], BF, tag="xf")
                nc.vector.tensor_tensor(b3(Xf), b3(X), bcR(st["f0"]), ALU.mult)
                st["Xf"] = Xf
                return
            AT = st["AT"]
            psD = ps_f.tile([128, W], FP, tag="mmf")
            for h in (0, 1):
                for r in range(R):
                    nc.tensor.matmul(jbh(psD, h, r), jbh(XT, h, r),
                                     jbh(AT, h, r))
            q = work.tile([128, W], FP, tag="ffq")
            nc.scalar.activation(q[:, :], psD[:, :], AF.Square)
            Dd = work.tile([128, W], BF, tag="dd")
            nc.scalar.copy(Dd[:, :], psD[:, :])
            u = work.tile([128, W], FP, tag="ffu")
            eng("u").tensor_scalar(u[:, :], q[:, :], -1.0, 1.0 + EPS_U,
                                   ALU.mult, ALU.add)
            rr = quake(work, u[:, :], [128, W], "ff", BF)
            zs = work.tile([128, W], BF, tag="zs")
            nc.vector.tensor_tensor(zs[:, :], Dd[:, :], rr[:, :], ALU.mult)
            th = work.tile([128, W], BF, tag="th")
            nc.scalar.activation(th[:, :], zs[:, :], AF.Arctan, scale=-1.0)
            thp = work.tile([128, W], BF, tag="thp")
            nc.vector.tensor_scalar(thp[:, :], th[:, :], HALF_PI, None,
                                    ALU.add)
            f = work.tile([128, W], BF, tag="fv")
            nc.vector.tensor_tensor(f[:, :], thp[:, :], rr[:, :], ALU.mult)
            S = deep.tile([128, W], BF, tag="sg")
            nc.vector.tensor_tensor(S[:, :], w_g[:, :], f[:, :], ALU.mult)
            st["S"] = S

        def emit_gmm(st, it):
            X = st["X"]
            psG = ps_f.tile([128, W], FP, tag="mmf")
            if it == 0:
                Xf = st["Xf"]
                for h in (0, 1):
                    for c in (0, 512):
                        nc.tensor.matmul(psG[64 * h:64 * h + 64, c:c + 512],
                                         w_g[64 * h:64 * h + 64, 0:64],
                                         Xf[64 * h:64 * h + 64, c:c + 512])
            else:
                S = st["S"]
                for h in (0, 1):
                    for r in range(R):
                        nc.tensor.matmul(jbh(psG, h, r), jbh(S, h, r),
                                         jbh(X, h, r))
            Gd = deep.tile([128, W], BF, tag="gd")
            nc.scalar.copy(Gd[:, :], psG[:, :])
            # write this block's prod/g2 slices now so psG frees in ACT order
            pg, idx = st["pg"], st["pgidx"]
            off = 2 * W * idx
            nc.scalar.activation(pg[:, off + W:off + 2 * W],
                                 psG[:, :], AF.Square)
            eng("prod").tensor_tensor(pg[:, off:off + W], st["A"][:, :],
                                      Gd[:, :], ALU.mult)
            st["Gd"] = Gd

        def emit_update(sts, it):
            npair = len(sts)
            pg = sts[0]["pg"]
            nred = 2 * R * npair
            red = red_p.tile([128, nred], FP, tag="red")
            rview = pg[:, :].rearrange("p (s d) -> p s d", d=64)
            for c0 in range(0, npair, 2):
                seg = slice(2 * R * c0, 2 * R * (c0 + 2))
                nc.vector.tensor_reduce(
                    red[:, seg].rearrange("p (s j) -> p s j", j=R),
                    rview[:, 2 * c0 * 16:2 * (c0 + 2) * 16], AX.X, ALU.add)
            # red cols: [idx][kind][j]: coefA at kind 0, gnr at kind 1
            rv = red[:, :].rearrange("p (i k j) -> p i k j", k=2, j=R)
            shape = [128, R * npair]
            coefA = red[:, :].rearrange("p (i k j) -> p (i k) j", k=2, j=R)
            # strided views
            cview = rv[:, :, 0, :]          # [128, npair, R]
            gview = rv[:, :, 1, :]
            gn2 = red_p.tile(shape, FP, tag="gn2")
            g3 = gn2[:, :].rearrange("p (i j) -> p i j", j=R)
            nc.vector._custom_dve(GN2_F, out=g3, in0=gview, in1=cview,
                                  s0=1e-30)
            rg = quake(red_p, gn2[:, :], shape, "rg", FP, nr=True)
            gn = red_p.tile(shape, FP, tag="gn")
            nc.vector.tensor_tensor(gn[:, :], gn2[:, :], rg[:, :], ALU.mult)
            cosg = red_p.tile(shape, FP, tag="cosg")
            nc.scalar.activation(cosg[:, :], gn[:, :], AF.Sin,
                                 bias=halfpi[:, 0:1])
            s1t = red_p.tile(shape, FP, tag="s1t")
            nc.scalar.activation(s1t[:, :], gn[:, :], AF.Sin)
            sc = red_p.tile(shape, FP, tag="sc")
            nc.vector.tensor_tensor(sc[:, :], s1t[:, :], rg[:, :], ALU.mult)
            t9 = red_p.tile(shape, FP, tag="t9")
            nc.vector.scalar_tensor_tensor(
                t9[:, :].rearrange("p (i j) -> p i j", j=R), sc[:, :]
                .rearrange("p (i j) -> p i j", j=R), -1.0, cview,
                ALU.mult, ALU.mult)
            alpha = red_p.tile(shape, BF, tag="alpha")
            nc.vector.tensor_tensor(alpha[:, :], cosg[:, :], t9[:, :], ALU.add)
            scb = red_p.tile(shape, BF, tag="scb")
            nc.vector.tensor_copy(scb[:, :], sc[:, :])
            last = it == 2
            for idx, st in enumerate(sts):
                A, Gd = st["A"], st["Gd"]
                t1 = work.tile([128, W], BF, tag="scr1")
                eng("t1").tensor_tensor(b3(t1), b3(A),
                                        bcR(alpha[:, R * idx:R * idx + R]),
                                        ALU.mult)
                t2 = work.tile([128, W], BF, tag="scr2")
                eng("t2").tensor_tensor(b3(t2), b3(Gd),
                                        bcR(scb[:, R * idx:R * idx + R]),
                                        ALU.mult)
                An = ab_p.tile([128, W], FP if last else BF,
                               tag="agf" if last else "ag")
                nc.vector.tensor_tensor(An[:, :], t1[:, :], t2[:, :], ALU.add)
                st["A"] = An
                if not last:
                    tp = ps_t.tile([128, W], BF, tag="tp")
                    for h in (0, 1):
                        for r in range(R):
                            nc.tensor.transpose(jbh(tp, h, r), jbh(An, h, r),
                                                ident[64 * h:64 * h + 64, :])
                    AT = ab_p.tile([128, W], BF, tag="at")
                    nc.scalar.copy(AT[:, :], tp[:, :])
                    st["AT"] = AT
                else:
                    for h in (0, 1):
                        nc.sync.dma_start(
                            out_d[st["b"], h],
                            b3(An[64 * h:64 * h + 64, :]))

        all_sts = [{"b": b} for b in range(NBLK)]
        for st in all_sts:
            emit_load(st)
            emit_d0(st)
        for b0 in range(0, NBLK, INTERLEAVE):
            sts = all_sts[b0:b0 + INTERLEAVE]
            for it in range(3):
                for st in sts:
                    emit_factor(st, it)
                pg = work.tile([128, 2 * W * len(sts)], BF, tag="pg")
                for idx, st in enumerate(sts):
                    st["pg"], st["pgidx"] = pg, idx
                    emit_gmm(st, it)
                emit_update(sts, it)
    nc.compile()
    return nc


def _get_program(**kw):
    key = tuple(sorted((k, tuple(v) if isinstance(v, (list, tuple, set, frozenset))
                        else v) for k, v in kw.items()))
    if key not in _COMPILED:
        _COMPILED[key] = build_program(**kw)
    return _COMPILED[key]


def kernel(x, w_raw, _trace=False, **bkw):
    import ml_dtypes
    from concourse.bass_utils import run_bass_kernel_spmd
    if _trace:
        _trace = _ensure_trace_hook()

    bf16 = ml_dtypes.bfloat16
    x = np.asarray(x, f32)
    w_raw = np.asarray(w_raw, f32)
    B, L, C_in, d = x.shape
    N = B * L
    w = np.exp((w_raw - f32(np.log(C_in))).astype(f32)).astype(f32)
    w = (w / w.sum(axis=0, keepdims=True)).astype(f32)

    xr = x.reshape(N, C_in, d)
    # per core: [NBLK, 2, R, i, d] -> transpose to [NBLK, 2, i, R, d]
    xcore = xr.reshape(N_CORES, NBLK, 2, R, C_in, d)
    xp = np.ascontiguousarray(xcore.transpose(0, 1, 2, 4, 3, 5)).astype(bf16)
    x0p = np.ascontiguousarray(xcore[:, :, :, :, 0, :]).astype(bf16)
    w_rep = np.ascontiguousarray(
        np.broadcast_to(w.T.reshape(1, 64, 1, 64), (2, 64, R, 64))
        .transpose(0, 3, 2, 1).reshape(128, W)).astype(bf16)
    # w_rep[p, (j, o)]: lower/upper halves identical, = w[i=p%64, o]
    w_rep = np.ascontiguousarray(
        np.tile(np.repeat(w[None, :, :], 1, axis=0), (2, 1, 1))  # (2,64,64)
        .reshape(2, 64, 1, 64).repeat(R, axis=2).reshape(2 * 64, R * 64)
        ).astype(bf16)
    ident2 = np.tile(np.eye(64, dtype=bf16), (2, 1))

    nc = _get_program(**bkw)
    in_maps = []
    for k in range(N_CORES):
        in_maps.append({
            "xp": xp[k],
            "x0p": x0p[k],
            "w_rep": w_rep,
            "ident2": ident2,
        })
    res = run_bass_kernel_spmd(nc, in_maps, core_ids=list(range(N_CORES)),
                               trace=_trace)
    # out_p: [NBLK, 2, o, j, d] per core -> rows
    outs = []
    for k in range(N_CORES):
        op = res.results[k]["out_p"]          # (NBLK, 2, 64, R, 64)
        outs.append(np.ascontiguousarray(op.transpose(0, 1, 3, 2, 4))
                    .reshape(ROWS_PER_CORE, C_OUT, d))
    out = np.concatenate(outs, axis=0)
    if _trace:
        kernel.last_exec_time_ns = res.exec_time_ns
        kernel.last_results = res
    return out.reshape(B, L, C_OUT, d).astype(f32)


# revision 5
# speedup vs baseline: 2.2505x; 1.0090x over previous
"""Trainium2 Bass kernel for nn_MfdFC (spherical weighted-Frechet-mean).

Math per row n (N=1024, 128 rows/core): w = col-softmax(w_raw); a(o) <- x0;
3 iterations of  D = <a_o, x_i>;  f = (pi/2 + arctan(-D*rr))*rr with
rr = rsqrt(1+eps-D^2) (quake rsqrt on DVE, arctan on ACT);  S = w^T。f;
G = S @ X;  c = sum_d A。G;  gn = sqrt(sum G^2 - c^2);
a <- (cos gn - sinc(gn) c) a + sinc(gn) G.

Design: all matmuls bf16 (4x PE rate), elementwise mostly bf16 (2x DVE).
128 rows/core as 4 blocks of 32, halves stacked at PSUM partitions 0-63 /
64-127 (PE writes upper partitions directly). All FOUR blocks pipelined
(INTERLEAVE=4); long-lived tiles (Gd/S/Xf/An/AT) sit in deep-rotation pools
so pool reuse cannot cycle with the in-order engine queues. Host pre-
transposes x (and pre-replicates w) so every DMA is contiguous; iteration 0
is specialized (a==x0: [128,16] f-chain, 2 wide matmuls for G0); the
update-phase reductions are fused 4096-wide; q is emitted before Dd so ACT
serves the longer DVE chain first.
"""
import math
import numpy as np

f32 = np.float32

C_IN = 64
C_OUT = 64
D_DIM = 64
ROWS_PER_CORE = 128
N_CORES = 8
R = 16
NBLK = 4
W = 64 * R
RSQ_C1 = 1.7584694439735017e-30
RSQ_C2 = -2.755803843779718e-20
HALF_PI = float(f32(math.pi / 2.0))
EPS_U = float(f32(2.0 ** -22))

_COMPILED = {}

def _register_custom_ops():
    import concourse.dve_ops as dve_ops
    from concourse.dve_ops import DveOp
    from concourse.dve_spec import (
        Spec, Src0, Src1, C0, C1, lower, maxx, _has_src1 as has_src1,
    )
    from concourse.dve_uop import DveOpSpec
    from concourse.dve_table_gen import dve_ver_for

    if "ANT_RSQ_F" in dve_ops._SUB_OPCODE_FOR_NAME:
        return {n: op for n, op in ((o.name, o) for o in dve_ops.OPS)
                if n.startswith("ANT_")}

    def _ref_rsq_f(in0, in1, s0, s1, imm2):
        u = np.asarray(in0, f32)
        nt = np.asarray(in1, f32)
        m1 = (nt * f32(s0)).astype(f32)
        m2 = (m1 * nt).astype(f32)
        m3 = (m2 * f32(s0)).astype(f32)
        t = (m3 * u).astype(f32)
        return ((t + f32(s1)) * nt).astype(f32)

    _m1 = Src1 * C0
    _m3 = (_m1 * Src1) * C0
    RSQ_F = DveOp("ANT_RSQ_F",
                  Spec(body=((_m3 * Src0) + C1) * Src1, reference=_ref_rsq_f),
                  subdim=False, uops_sha={})

    def _ref_rsq_nr(in0, in1, s0, s1, imm2):
        u = np.asarray(in0, f32); y = np.asarray(in1, f32)
        a = (u * y).astype(f32)
        b = (a * y).astype(f32)
        return ((f32(s0) - (b * f32(s1)).astype(f32)) * y).astype(f32)

    RSQ_NR = DveOp("ANT_RSQ_NR",
                   Spec(body=(C0 - ((Src0 * Src1) * Src1) * C1) * Src1,
                        reference=_ref_rsq_nr),
                   subdim=False, uops_sha={})

    def _ref_gn2(in0, in1, s0, s1, imm2):
        raw = np.asarray(in0, f32); c = np.asarray(in1, f32)
        return np.maximum((raw - (c * c).astype(f32)).astype(f32), f32(s0))

    GN2_F = DveOp("ANT_GN2_F",
                  Spec(body=maxx(Src0 - Src1 * Src1, C0), reference=_ref_gn2),
                  subdim=False, uops_sha={})

    ops = [RSQ_F, RSQ_NR, GN2_F]
    base = dve_ops._CUSTOM_DVE_ROW_BASE + len(dve_ops.OPS)
    for i, op in enumerate(ops):
        dve_ops._SUB_OPCODE_FOR_NAME[op.name] = base + i
    for trn in ("TRN2",):
        ver = dve_ver_for(trn)
        for op in ops:
            uops = lower(op.spec, ver=ver)
            s = DveOpSpec(name=op.name, opcode=dve_ops.get_dve_sub_opcode(op.name),
                          uops=uops, rd1_en=has_src1(op.spec))
            op.uops_sha[ver] = s.sha(ver)
    dve_ops.OPS.extend(ops)
    dve_ops.CUSTOM_DVE_SPECS.update({op.name: op.spec for op in ops})
    return {op.name: op for op in ops}



def _ensure_trace_hook():
    try:
        from antenv.axon_hooks import get_axon_ntff_profile_hook
        return get_axon_ntff_profile_hook() is not None
    except ImportError:
        pass
    try:
        import sys, types
        import antenv
        from trn_agent_boot.trn_boot import _ntff_profile_via_ctypes
        mod = types.ModuleType("antenv.axon_hooks")
        _h = {}
        mod.set_axon_ntff_profile_hook = lambda h: _h.__setitem__("h", h)
        mod.get_axon_ntff_profile_hook = lambda: _h.get("h")
        sys.modules["antenv.axon_hooks"] = mod
        antenv.axon_hooks = mod
        mod.set_axon_ntff_profile_hook(
            _ntff_profile_via_ctypes("/opt/axon/libaxon_pjrt.so"))
        return True
    except Exception:
        return False





def build_program(INTERLEAVE=4, gps=(), redsplit=False, wbufs=2):
    from contextlib import ExitStack
    import concourse.bacc as bacc
    import concourse.mybir as mybir
    import concourse.tile as tile

    gps = frozenset(gps)
    FP = mybir.dt.float32
    BF = mybir.dt.bfloat16
    I32 = mybir.dt.int32
    AF = mybir.ActivationFunctionType
    ALU = mybir.AluOpType
    AX = mybir.AxisListType

    OPS = _register_custom_ops()
    RSQ_F, RSQ_NR, GN2_F = OPS["ANT_RSQ_F"], OPS["ANT_RSQ_NR"], OPS["ANT_GN2_F"]

    nc = bacc.Bacc()
    # x pre-transposed on host: [block, half, i, j, d]
    x_d = nc.dram_tensor("xp", (NBLK, 2, C_IN, R, D_DIM), BF,
                         kind="ExternalInput")
    # x0 rows: [block, half, j, d]
    x0_d = nc.dram_tensor("x0p", (NBLK, 2, R, D_DIM), BF, kind="ExternalInput")
    w_d = nc.dram_tensor("w_rep", (128, W), BF, kind="ExternalInput")
    id_d = nc.dram_tensor("ident2", (128, 64), BF, kind="ExternalInput")
    # output in SBUF-natural order: [block, half, o, j, d]
    out_d = nc.dram_tensor("out_p", (NBLK, 2, C_OUT, R, D_DIM), FP,
                           kind="ExternalOutput")

    ctx = ExitStack()
    with ctx:
        tc = ctx.enter_context(tile.TileContext(nc))
        const = ctx.enter_context(tc.tile_pool(name="const", bufs=1))
        xg_p = ctx.enter_context(tc.tile_pool(name="xg", bufs=NBLK))
        work = ctx.enter_context(tc.tile_pool(name="work", bufs=wbufs))
        deep = ctx.enter_context(tc.tile_pool(name="deep", bufs=max(2, INTERLEAVE)))
        ab_p = ctx.enter_context(tc.tile_pool(name="ab", bufs=NBLK))
        red_p = ctx.enter_context(tc.tile_pool(name="red", bufs=max(2, INTERLEAVE)))
        ps_f = ctx.enter_context(tc.tile_pool(name="psf", bufs=2, space="PSUM"))
        ps_t = ctx.enter_context(tc.tile_pool(name="pst", bufs=2, space="PSUM"))
        ps_s = ctx.enter_context(tc.tile_pool(name="pss", bufs=2, space="PSUM"))

        def eng(name):
            return nc.gpsimd if name in gps else nc.vector

        # ---- constants (all contiguous DMAs)
        w_g = const.tile([128, W], BF, tag="wg")
        nc.sync.dma_start(w_g[:, :], w_d[:, :])
        ident = const.tile([128, 64], BF, tag="ident")
        nc.sync.dma_start(ident[:, :], id_d[:, :])
        halfpi = const.tile([128, 1], FP, tag="halfpi")
        nc.vector.memset(halfpi[:, :], HALF_PI)

        def jbh(t, h, j):
            return t[64 * h:64 * h + 64, 64 * j:64 * j + 64]

        def b3(t):
            return t[:, :].rearrange("p (j d) -> p j d", d=64)

        def bcR(small_ap):      # [128, R] ap -> broadcast (p, j, 64)
            return small_ap.rearrange("p (j o) -> p j o", o=1) \
                .broadcast_to([128, R, 64])

        def emit_load(st):
            b = st["b"]
            X = xg_p.tile([128, W], BF, tag="xg")
            for h in (0, 1):
                nc.sync.dma_start(b3(X[64 * h:64 * h + 64, :]), x_d[b, h])
            A0 = ab_p.tile([128, W], BF, tag="a0")
            for h in (0, 1):
                nc.sync.dma_start(
                    A0[64 * h:64 * h + 64, :].rearrange("p (j d) -> p j d", d=64),
                    x0_d[b:b + 1, h].rearrange("b j d -> b j d")
                    .broadcast_to([64, R, 64]))
            tp = ps_t.tile([128, W], BF, tag="tp")
            for h in (0, 1):
                for r in range(R):
                    nc.tensor.transpose(jbh(tp, h, r), jbh(X, h, r),
                                        ident[64 * h:64 * h + 64, :])
            XT = xg_p.tile([128, W], BF, tag="xt")
            nc.scalar.copy(XT[:, :], tp[:, :])
            st["X"], st["XT"], st["A"] = X, XT, A0

        def quake(pool, src_ap, shape, tagp, out_dt, nr=False):
            seed = pool.tile(shape, FP, tag=tagp + "sd")
            nc.vector.tensor_scalar(seed[:, :].bitcast(I32),
                                    src_ap.bitcast(I32), 1, -1,
                                    ALU.logical_shift_right, ALU.bitwise_xor)
            rr = pool.tile(shape, out_dt, tag=tagp + "rr")
            nc.vector._custom_dve(RSQ_F, out=rr[:, :], in0=src_ap,
                                  in1=seed[:, :], s0=RSQ_C1, s1=RSQ_C2)
            if not nr:
                return rr
            rr2 = pool.tile(shape, out_dt, tag=tagp + "r2")
            nc.vector._custom_dve(RSQ_NR, out=rr2[:, :], in0=src_ap,
                                  in1=rr[:, :], s0=1.5, s1=0.5)
            return rr2

        # ---------- iteration 0: per-block D0 + small f-chain
        def emit_d0(st):
            XT = st["XT"]
            psD0 = ps_s.tile([128, R], FP, tag="d0")
            for h in (0, 1):
                for r in range(R):
                    nc.tensor.matmul(
                        psD0[64 * h:64 * h + 64, r:r + 1],
                        jbh(XT, h, r),
                        XT[64 * h:64 * h + 64, 64 * r:64 * r + 1])
            shape = [128, R]
            q0 = red_p.tile(shape, FP, tag="f0q")
            nc.scalar.activation(q0[:, :], psD0[:, :], AF.Square)
            u0 = red_p.tile(shape, FP, tag="f0u")
            nc.vector.tensor_scalar(u0[:, :], q0[:, :], -1.0, 1.0 + EPS_U,
                                    ALU.mult, ALU.add)
            rr0 = quake(red_p, u0[:, :], shape, "f0", FP)
            zs0 = red_p.tile(shape, FP, tag="f0z")
            nc.vector.tensor_tensor(zs0[:, :], psD0[:, :], rr0[:, :], ALU.mult)
            th0 = red_p.tile(shape, FP, tag="f0t")
            nc.scalar.activation(th0[:, :], zs0[:, :], AF.Arctan, scale=-1.0)
            f0 = red_p.tile(shape, BF, tag="f0v")
            nc.vector.scalar_tensor_tensor(f0[:, :], th0[:, :], HALF_PI,
                                           rr0[:, :], ALU.add, ALU.mult)
            st["f0"] = f0[:, :]

        def emit_factor(st, it):
            X, XT = st["X"], st["XT"]
            if it == 0:
                Xf = deep.tile([128, W], BF, tag="xf")
                nc.vector.tensor_tensor(b3(Xf), b3(X), bcR(st["f0"]), ALU.mult)
                st["Xf"] = Xf
                return
            AT = st["AT"]
            psD = ps_f.tile([128, W], FP, tag="mmf")
            for h in (0, 1):
                for r in range(R):
                    nc.tensor.matmul(jbh(psD, h, r), jbh(XT, h, r),
                                     jbh(AT, h, r))
            q = work.tile([128, W], FP, tag="ffq")
            nc.scalar.activation(q[:, :], psD[:, :], AF.Square)
            Dd = work.tile([128, W], BF, tag="dd")
            nc.scalar.copy(Dd[:, :], psD[:, :])
            u = work.tile([128, W], FP, tag="ffu")
            eng("u").tensor_scalar(u[:, :], q[:, :], -1.0, 1.0 + EPS_U,
                                   ALU.mult, ALU.add)
            rr = quake(work, u[:, :], [128, W], "ff", BF)
            zs = work.tile([128, W], BF, tag="zs")
            nc.vector.tensor_tensor(zs[:, :], Dd[:, :], rr[:, :], ALU.mult)
            th = work.tile([128, W], BF, tag="th")
            nc.scalar.activation(th[:, :], zs[:, :], AF.Arctan, scale=-1.0)
            thp = work.tile([128, W], BF, tag="thp")
            nc.vector.tensor_scalar(thp[:, :], th[:, :], HALF_PI, None,
                                    ALU.add)
            f = work.tile([128, W], BF, tag="fv")
            nc.vector.tensor_tensor(f[:, :], thp[:, :], rr[:, :], ALU.mult)
            S = deep.tile([128, W], BF, tag="sg")
            nc.vector.tensor_tensor(S[:, :], w_g[:, :], f[:, :], ALU.mult)
            st["S"] = S

        def emit_gmm(st, it):
            X = st["X"]
            psG = ps_f.tile([128, W], FP, tag="mmf")
            if it == 0:
                Xf = st["Xf"]
                for h in (0, 1):
                    for c in (0, 512):
                        nc.tensor.matmul(psG[64 * h:64 * h + 64, c:c + 512],
                                         w_g[64 * h:64 * h + 64, 0:64],
                                         Xf[64 * h:64 * h + 64, c:c + 512])
            else:
                S = st["S"]
                for h in (0, 1):
                    for r in range(R):
                        nc.tensor.matmul(jbh(psG, h, r), jbh(S, h, r),
                                         jbh(X, h, r))
            Gd = deep.tile([128, W], BF, tag="gd")
            nc.scalar.copy(Gd[:, :], psG[:, :])
            # write this block's prod/g2 slices now so psG frees in ACT order
            pg, idx = st["pg"], st["pgidx"]
            off = 2 * W * idx
            nc.scalar.activation(pg[:, off + W:off + 2 * W],
                                 psG[:, :], AF.Square)
            eng("prod").tensor_tensor(pg[:, off:off + W], st["A"][:, :],
                                      Gd[:, :], ALU.mult)
            st["Gd"] = Gd

        def emit_update(sts, it):
            npair = len(sts)
            pg = sts[0]["pg"]
            nred = 2 * R * npair
            red = red_p.tile([128, nred], FP, tag="red")
            rview = pg[:, :].rearrange("p (s d) -> p s d", d=64)
            for c0 in range(0, npair, 2):
                seg = slice(2 * R * c0, 2 * R * (c0 + 2))
                nc.vector.tensor_reduce(
                    red[:, seg].rearrange("p (s j) -> p s j", j=R),
                    rview[:, 2 * c0 * 16:2 * (c0 + 2) * 16], AX.X, ALU.add)
            # red cols: [idx][kind][j]: coefA at kind 0, gnr at kind 1
            rv = red[:, :].rearrange("p (i k j) -> p i k j", k=2, j=R)
            shape = [128, R * npair]
            coefA = red[:, :].rearrange("p (i k j) -> p (i k) j", k=2, j=R)
            # strided views
            cview = rv[:, :, 0, :]          # [128, npair, R]
            gview = rv[:, :, 1, :]
            gn2 = red_p.tile(shape, FP, tag="gn2")
            g3 = gn2[:, :].rearrange("p (i j) -> p i j", j=R)
            nc.vector._custom_dve(GN2_F, out=g3, in0=gview, in1=cview,
                                  s0=1e-30)
            rg = quake(red_p, gn2[:, :], shape, "rg", FP, nr=True)
            gn = red_p.tile(shape, FP, tag="gn")
            nc.vector.tensor_tensor(gn[:, :], gn2[:, :], rg[:, :], ALU.mult)
            cosg = red_p.tile(shape, FP, tag="cosg")
            nc.scalar.activation(cosg[:, :], gn[:, :], AF.Sin,
                                 bias=halfpi[:, 0:1])
            s1t = red_p.tile(shape, FP, tag="s1t")
            nc.scalar.activation(s1t[:, :], gn[:, :], AF.Sin)
            sc = red_p.tile(shape, FP, tag="sc")
            nc.vector.tensor_tensor(sc[:, :], s1t[:, :], rg[:, :], ALU.mult)
            t9 = red_p.tile(shape, FP, tag="t9")
            nc.vector.scalar_tensor_tensor(
                t9[:, :].rearrange("p (i j) -> p i j", j=R), sc[:, :]
                .rearrange("p (i j) -> p i j", j=R), -1.0, cview,
                ALU.mult, ALU.mult)
            alpha = red_p.tile(shape, BF, tag="alpha")
            nc.vector.tensor_tensor(alpha[:, :], cosg[:, :], t9[:, :], ALU.add)
            scb = red_p.tile(shape, BF, tag="scb")
            nc.vector.tensor_copy(scb[:, :], sc[:, :])
            last = it == 2
            for idx, st in enumerate(sts):
                A, Gd = st["A"], st["Gd"]
                t1 = work.tile([128, W], BF, tag="scr1")
                eng("t1").tensor_tensor(b3(t1), b3(A),
                                        bcR(alpha[:, R * idx:R * idx + R]),
                                        ALU.mult)
                t2 = work.tile([128, W], BF, tag="scr2")
                eng("t2").tensor_tensor(b3(t2), b3(Gd),
                                        bcR(scb[:, R * idx:R * idx + R]),
                                        ALU.mult)
                An = ab_p.tile([128, W], FP if last else BF,
                               tag="agf" if last else "ag")
                nc.vector.tensor_tensor(An[:, :], t1[:, :], t2[:, :], ALU.add)
                st["A"] = An
                if not last:
                    tp = ps_t.tile([128, W], BF, tag="tp")
                    for h in (0, 1):
                        for r in range(R):
                            nc.tensor.transpose(jbh(tp, h, r), jbh(An, h, r),
                                                ident[64 * h:64 * h + 64, :])
                    AT = ab_p.tile([128, W], BF, tag="at")
                    nc.scalar.copy(AT[:, :], tp[:, :])
                    st["AT"] = AT
                else:
                    for h in (0, 1):
                        nc.sync.dma_start(
                            out_d[st["b"], h],
                            b3(An[64 * h:64 * h + 64, :]))

        all_sts = [{"b": b} for b in range(NBLK)]
        for st in all_sts:
            emit_load(st)
            emit_d0(st)
        for b0 in range(0, NBLK, INTERLEAVE):
            sts = all_sts[b0:b0 + INTERLEAVE]
            for it in range(3):
                for st in sts:
                    emit_factor(st, it)
                pg = work.tile([128, 2 * W * len(sts)], BF, tag="pg")
                for idx, st in enumerate(sts):
                    st["pg"], st["pgidx"] = pg, idx
                    emit_gmm(st, it)
                emit_update(sts, it)
    nc.compile()
    return nc


def _get_program(**kw):
    key = tuple(sorted((k, tuple(v) if isinstance(v, (list, tuple, set, frozenset))
                        else v) for k, v in kw.items()))
    if key not in _COMPILED:
        _COMPILED[key] = build_program(**kw)
    return _COMPILED[key]


def kernel(x, w_raw, _trace=False, **bkw):
    import ml_dtypes
    from concourse.bass_utils import run_bass_kernel_spmd
    if _trace:
        _trace = _ensure_trace_hook()

    bf16 = ml_dtypes.bfloat16
    x = np.asarray(x, f32)
    w_raw = np.asarray(w_raw, f32)
    B, L, C_in, d = x.shape
    N = B * L
    w = np.exp((w_raw - f32(np.log(C_in))).astype(f32)).astype(f32)
    w = (w / w.sum(axis=0, keepdims=True)).astype(f32)

    xr = x.reshape(N, C_in, d)
    # per core: [NBLK, 2, R, i, d] -> transpose to [NBLK, 2, i, R, d]
    xcore = xr.reshape(N_CORES, NBLK, 2, R, C_in, d)
    xp = np.ascontiguousarray(xcore.transpose(0, 1, 2, 4, 3, 5)).astype(bf16)
    x0p = np.ascontiguousarray(xcore[:, :, :, :, 0, :]).astype(bf16)
    w_rep = np.ascontiguousarray(
        np.broadcast_to(w.T.reshape(1, 64, 1, 64), (2, 64, R, 64))
        .transpose(0, 3, 2, 1).reshape(128, W)).astype(bf16)
    # w_rep[p, (j, o)]: lower/upper halves identical, = w[i=p%64, o]
    w_rep = np.ascontiguousarray(
        np.tile(np.repeat(w[None, :, :], 1, axis=0), (2, 1, 1))  # (2,64,64)
        .reshape(2, 64, 1, 64).repeat(R, axis=2).reshape(2 * 64, R * 64)
        ).astype(bf16)
    ident2 = np.tile(np.eye(64, dtype=bf16), (2, 1))

    nc = _get_program(**bkw)
    in_maps = []
    for k in range(N_CORES):
        in_maps.append({
            "xp": xp[k],
            "x0p": x0p[k],
            "w_rep": w_rep,
            "ident2": ident2,
        })
    res = run_bass_kernel_spmd(nc, in_maps, core_ids=list(range(N_CORES)),
                               trace=_trace)
    # out_p: [NBLK, 2, o, j, d] per core -> rows
    outs = []
    for k in range(N_CORES):
        op = res.results[k]["out_p"]          # (NBLK, 2, 64, R, 64)
        outs.append(np.ascontiguousarray(op.transpose(0, 1, 3, 2, 4))
                    .reshape(ROWS_PER_CORE, C_OUT, d))
    out = np.concatenate(outs, axis=0)
    if _trace:
        kernel.last_exec_time_ns = res.exec_time_ns
        kernel.last_results = res
    return out.reshape(B, L, C_OUT, d).astype(f32)


# revision 6
# speedup vs baseline: 2.3873x; 1.0608x over previous
"""Trainium2 Bass kernel v3 for nn_MfdFC. See kernel_v2 docstring for math.

v3 over v2:
- host pre-transposes x into per-(block,half) [i, j, d] layout and w into the
  replicated [128, 1024] SBUF image -> all input DMAs are contiguous; the
  output is written in SBUF-natural [o, j, d] order and re-transposed on host.
- the per-block [128,16] "smalls" pipelines (iter-0 f-chain, update-phase
  cos/sin/rsqrt chain) run once per interleave-PAIR on [128,32] tiles, and
  the coefA/|G|^2 reductions of a pair are fused into ONE 4096-wide reduce.
- optional GPSIMD offload for selected elementwise passes (t2, prod).
"""
import math
import numpy as np

f32 = np.float32

C_IN = 64
C_OUT = 64
D_DIM = 64
ROWS_PER_CORE = 128
N_CORES = 8
R = 16
NBLK = 4
W = 64 * R
RSQ_C1 = 1.7584694439735017e-30
RSQ_C2 = -2.755803843779718e-20
HALF_PI = float(f32(math.pi / 2.0))
EPS_U = float(f32(2.0 ** -22))

_COMPILED = {}

def _register_custom_ops():
    import concourse.dve_ops as dve_ops
    from concourse.dve_ops import DveOp
    from concourse.dve_spec import (
        Spec, Src0, Src1, C0, C1, lower, maxx, _has_src1 as has_src1,
    )
    from concourse.dve_uop import DveOpSpec
    from concourse.dve_table_gen import dve_ver_for

    if "ANT_RSQ_F" in dve_ops._SUB_OPCODE_FOR_NAME:
        return {n: op for n, op in ((o.name, o) for o in dve_ops.OPS)
                if n.startswith("ANT_")}

    def _ref_rsq_f(in0, in1, s0, s1, imm2):
        u = np.asarray(in0, f32)
        nt = np.asarray(in1, f32)
        m1 = (nt * f32(s0)).astype(f32)
        m2 = (m1 * nt).astype(f32)
        m3 = (m2 * f32(s0)).astype(f32)
        t = (m3 * u).astype(f32)
        return ((t + f32(s1)) * nt).astype(f32)

    _m1 = Src1 * C0
    _m3 = (_m1 * Src1) * C0
    RSQ_F = DveOp("ANT_RSQ_F",
                  Spec(body=((_m3 * Src0) + C1) * Src1, reference=_ref_rsq_f),
                  subdim=False, uops_sha={})

    def _ref_rsq_nr(in0, in1, s0, s1, imm2):
        u = np.asarray(in0, f32); y = np.asarray(in1, f32)
        a = (u * y).astype(f32)
        b = (a * y).astype(f32)
        return ((f32(s0) - (b * f32(s1)).astype(f32)) * y).astype(f32)

    RSQ_NR = DveOp("ANT_RSQ_NR",
                   Spec(body=(C0 - ((Src0 * Src1) * Src1) * C1) * Src1,
                        reference=_ref_rsq_nr),
                   subdim=False, uops_sha={})

    def _ref_gn2(in0, in1, s0, s1, imm2):
        raw = np.asarray(in0, f32); c = np.asarray(in1, f32)
        return np.maximum((raw - (c * c).astype(f32)).astype(f32), f32(s0))

    GN2_F = DveOp("ANT_GN2_F",
                  Spec(body=maxx(Src0 - Src1 * Src1, C0), reference=_ref_gn2),
                  subdim=False, uops_sha={})

    ops = [RSQ_F, RSQ_NR, GN2_F]
    base = dve_ops._CUSTOM_DVE_ROW_BASE + len(dve_ops.OPS)
    for i, op in enumerate(ops):
        dve_ops._SUB_OPCODE_FOR_NAME[op.name] = base + i
    for trn in ("TRN2",):
        ver = dve_ver_for(trn)
        for op in ops:
            uops = lower(op.spec, ver=ver)
            s = DveOpSpec(name=op.name, opcode=dve_ops.get_dve_sub_opcode(op.name),
                          uops=uops, rd1_en=has_src1(op.spec))
            op.uops_sha[ver] = s.sha(ver)
    dve_ops.OPS.extend(ops)
    dve_ops.CUSTOM_DVE_SPECS.update({op.name: op.spec for op in ops})
    return {op.name: op for op in ops}



def _ensure_trace_hook():
    try:
        from antenv.axon_hooks import get_axon_ntff_profile_hook
        return get_axon_ntff_profile_hook() is not None
    except ImportError:
        pass
    try:
        import sys, types
        import antenv
        from trn_agent_boot.trn_boot import _ntff_profile_via_ctypes
        mod = types.ModuleType("antenv.axon_hooks")
        _h = {}
        mod.set_axon_ntff_profile_hook = lambda h: _h.__setitem__("h", h)
        mod.get_axon_ntff_profile_hook = lambda: _h.get("h")
        sys.modules["antenv.axon_hooks"] = mod
        antenv.axon_hooks = mod
        mod.set_axon_ntff_profile_hook(
            _ntff_profile_via_ctypes("/opt/axon/libaxon_pjrt.so"))
        return True
    except Exception:
        return False





def build_program(INTERLEAVE=4, gps=(), redsplit=False, wbufs=2,
                  stagger=False, psf=2, pst=2, pss=2, dbufs=None, fold2=True, fold3=False):
    from contextlib import ExitStack
    import concourse.bacc as bacc
    import concourse.mybir as mybir
    import concourse.tile as tile

    gps = frozenset(gps)
    FP = mybir.dt.float32
    BF = mybir.dt.bfloat16
    I32 = mybir.dt.int32
    AF = mybir.ActivationFunctionType
    ALU = mybir.AluOpType
    AX = mybir.AxisListType

    OPS = _register_custom_ops()
    RSQ_F, RSQ_NR, GN2_F = OPS["ANT_RSQ_F"], OPS["ANT_RSQ_NR"], OPS["ANT_GN2_F"]

    nc = bacc.Bacc()
    # x pre-transposed on host: [block, half, i, j, d]
    x_d = nc.dram_tensor("xp", (NBLK, 2, C_IN, R, D_DIM), BF,
                         kind="ExternalInput")
    # x0 rows: [block, half, j, d]
    x0_d = nc.dram_tensor("x0p", (NBLK, 2, R, D_DIM), BF, kind="ExternalInput")
    w_d = nc.dram_tensor("w_rep", (128, W), BF, kind="ExternalInput")
    id_d = nc.dram_tensor("ident2", (128, 64), BF, kind="ExternalInput")
    # output in SBUF-natural order: [block, half, o, j, d]
    out_d = nc.dram_tensor("out_p", (NBLK, 2, C_OUT, R, D_DIM), FP,
                           kind="ExternalOutput")

    ctx = ExitStack()
    with ctx:
        tc = ctx.enter_context(tile.TileContext(nc))
        const = ctx.enter_context(tc.tile_pool(name="const", bufs=1))
        xg_p = ctx.enter_context(tc.tile_pool(name="xg", bufs=NBLK))
        work = ctx.enter_context(tc.tile_pool(name="work", bufs=wbufs))
        deep = ctx.enter_context(tc.tile_pool(name="deep", bufs=dbufs or max(2, INTERLEAVE)))
        ab_p = ctx.enter_context(tc.tile_pool(name="ab", bufs=NBLK))
        red_p = ctx.enter_context(tc.tile_pool(name="red", bufs=max(2, INTERLEAVE)))
        ps_f = ctx.enter_context(tc.tile_pool(name="psf", bufs=psf, space="PSUM"))
        ps_t = ctx.enter_context(tc.tile_pool(name="pst", bufs=pst, space="PSUM"))
        ps_s = ctx.enter_context(tc.tile_pool(name="pss", bufs=pss, space="PSUM"))

        def eng(name):
            return nc.gpsimd if name in gps else nc.vector

        # ---- constants (all contiguous DMAs)
        w_g = const.tile([128, W], BF, tag="wg")
        nc.sync.dma_start(w_g[:, :], w_d[:, :])
        ident = const.tile([128, 64], BF, tag="ident")
        nc.sync.dma_start(ident[:, :], id_d[:, :])
        halfpi = const.tile([128, 1], FP, tag="halfpi")
        nc.vector.memset(halfpi[:, :], HALF_PI)

        def jbh(t, h, j):
            return t[64 * h:64 * h + 64, 64 * j:64 * j + 64]

        def b3(t):
            return t[:, :].rearrange("p (j d) -> p j d", d=64)

        def bcR(small_ap):      # [128, R] ap -> broadcast (p, j, 64)
            return small_ap.rearrange("p (j o) -> p j o", o=1) \
                .broadcast_to([128, R, 64])

        def emit_load(st):
            b = st["b"]
            X = xg_p.tile([128, W], BF, tag="xg")
            for h in (0, 1):
                nc.sync.dma_start(b3(X[64 * h:64 * h + 64, :]), x_d[b, h])
            A0 = ab_p.tile([128, W], BF, tag="a0")
            for h in (0, 1):
                nc.sync.dma_start(
                    A0[64 * h:64 * h + 64, :].rearrange("p (j d) -> p j d", d=64),
                    x0_d[b:b + 1, h].rearrange("b j d -> b j d")
                    .broadcast_to([64, R, 64]))
            tp = ps_t.tile([128, W], BF, tag="tp")
            for h in (0, 1):
                for r in range(R):
                    nc.tensor.transpose(jbh(tp, h, r), jbh(X, h, r),
                                        ident[64 * h:64 * h + 64, :])
            XT = xg_p.tile([128, W], BF, tag="xt")
            nc.scalar.copy(XT[:, :], tp[:, :])
            st["X"], st["XT"], st["A"] = X, XT, A0

        def quake(pool, src_ap, shape, tagp, out_dt, nr=False):
            seed = pool.tile(shape, FP, tag=tagp + "sd")
            nc.vector.tensor_scalar(seed[:, :].bitcast(I32),
                                    src_ap.bitcast(I32), 1, -1,
                                    ALU.logical_shift_right, ALU.bitwise_xor)
            rr = pool.tile(shape, out_dt, tag=tagp + "rr")
            nc.vector._custom_dve(RSQ_F, out=rr[:, :], in0=src_ap,
                                  in1=seed[:, :], s0=RSQ_C1, s1=RSQ_C2)
            if not nr:
                return rr
            rr2 = pool.tile(shape, out_dt, tag=tagp + "r2")
            nc.vector._custom_dve(RSQ_NR, out=rr2[:, :], in0=src_ap,
                                  in1=rr[:, :], s0=1.5, s1=0.5)
            return rr2

        # ---------- iteration 0: per-block D0 + small f-chain
        def emit_d0(st):
            XT = st["XT"]
            psD0 = ps_s.tile([128, R], FP, tag="d0")
            for h in (0, 1):
                for r in range(R):
                    nc.tensor.matmul(
                        psD0[64 * h:64 * h + 64, r:r + 1],
                        jbh(XT, h, r),
                        XT[64 * h:64 * h + 64, 64 * r:64 * r + 1])
            shape = [128, R]
            q0 = red_p.tile(shape, FP, tag="f0q")
            nc.scalar.activation(q0[:, :], psD0[:, :], AF.Square)
            u0 = red_p.tile(shape, FP, tag="f0u")
            nc.vector.tensor_scalar(u0[:, :], q0[:, :], -1.0, 1.0 + EPS_U,
                                    ALU.mult, ALU.add)
            rr0 = quake(red_p, u0[:, :], shape, "f0", FP)
            zs0 = red_p.tile(shape, FP, tag="f0z")
            nc.vector.tensor_tensor(zs0[:, :], psD0[:, :], rr0[:, :], ALU.mult)
            th0 = red_p.tile(shape, FP, tag="f0t")
            nc.scalar.activation(th0[:, :], zs0[:, :], AF.Arctan, scale=-1.0)
            f0 = red_p.tile(shape, BF, tag="f0v")
            nc.vector.scalar_tensor_tensor(f0[:, :], th0[:, :], HALF_PI,
                                           rr0[:, :], ALU.add, ALU.mult)
            st["f0"] = f0[:, :]

        def emit_factor(st, it):
            X, XT = st["X"], st["XT"]
            if it == 0:
                Xf = deep.tile([128, W], BF, tag="xf")
                nc.vector.tensor_tensor(b3(Xf), b3(X), bcR(st["f0"]), ALU.mult)
                st["Xf"] = Xf
                return
            AT = st["AT"]
            psD = ps_f.tile([128, W], FP, tag="mmf")
            for h in (0, 1):
                for r in range(R):
                    nc.tensor.matmul(jbh(psD, h, r), jbh(XT, h, r),
                                     jbh(AT, h, r))
            q = work.tile([128, W], FP, tag="ffq")
            nc.scalar.activation(q[:, :], psD[:, :], AF.Square)
            Dd = work.tile([128, W], BF, tag="dd")
            nc.scalar.copy(Dd[:, :], psD[:, :])
            u = work.tile([128, W], FP, tag="ffu")
            eng("u").tensor_scalar(u[:, :], q[:, :], -1.0, 1.0 + EPS_U,
                                   ALU.mult, ALU.add)
            rr = quake(work, u[:, :], [128, W], "ff", BF)
            zs = work.tile([128, W], BF, tag="zs")
            nc.vector.tensor_tensor(zs[:, :], Dd[:, :], rr[:, :], ALU.mult)
            th = work.tile([128, W], BF, tag="th")
            nc.scalar.activation(th[:, :], zs[:, :], AF.Arctan, scale=-1.0)
            thp = work.tile([128, W], BF, tag="thp")
            nc.vector.tensor_scalar(thp[:, :], th[:, :], HALF_PI, None,
                                    ALU.add)
            f = work.tile([128, W], BF, tag="fv")
            nc.vector.tensor_tensor(f[:, :], thp[:, :], rr[:, :], ALU.mult)
            S = deep.tile([128, W], BF, tag="sg")
            nc.vector.tensor_tensor(S[:, :], w_g[:, :], f[:, :], ALU.mult)
            st["S"] = S

        def emit_gmm(st, it):
            X = st["X"]
            psG = ps_f.tile([128, W], FP, tag="mmf")
            if it == 0:
                Xf = st["Xf"]
                for h in (0, 1):
                    for c in (0, 512):
                        nc.tensor.matmul(psG[64 * h:64 * h + 64, c:c + 512],
                                         w_g[64 * h:64 * h + 64, 0:64],
                                         Xf[64 * h:64 * h + 64, c:c + 512])
            else:
                S = st["S"]
                for h in (0, 1):
                    for r in range(R):
                        nc.tensor.matmul(jbh(psG, h, r), jbh(S, h, r),
                                         jbh(X, h, r))
            Gd = deep.tile([128, W], BF, tag="gd")
            nc.scalar.copy(Gd[:, :], psG[:, :])
            # write this block's prod/g2 slices now so psG frees in ACT order
            pg, idx = st["pg"], st["pgidx"]
            off = 2 * W * idx
            nc.scalar.activation(pg[:, off + W:off + 2 * W],
                                 psG[:, :], AF.Square)
            eng("prod").tensor_tensor(pg[:, off:off + W], st["A"][:, :],
                                      Gd[:, :], ALU.mult)
            st["Gd"] = Gd

        def emit_update(sts, it):
            npair = len(sts)
            pg = sts[0]["pg"]
            nred = 2 * R * npair
            red = red_p.tile([128, nred], FP, tag="red")
            # stage 1: pairwise fold at bf16 2x mode (halves reduce volume)
            fold = work.tile([128, W * npair], BF, tag="fold")
            pv = pg[:, :].rearrange("p (s two q) -> p s two q", two=2, q=32)
            nc.vector.tensor_tensor(
                fold[:, :].rearrange("p (s q) -> p s q", q=32),
                pv[:, :, 0, :], pv[:, :, 1, :], ALU.add)
            if fold2:
                fb = work.tile([128, W * npair // 2], BF, tag="fold2")
                fv2 = fold[:, :].rearrange("p (s two q) -> p s two q",
                                           two=2, q=16)
                nc.vector.tensor_tensor(
                    fb[:, :].rearrange("p (s q) -> p s q", q=16),
                    fv2[:, :, 0, :], fv2[:, :, 1, :], ALU.add)
                if fold3:
                    fc = work.tile([128, W * npair // 4], BF, tag="fold3")
                    fv3 = fb[:, :].rearrange("p (s two q) -> p s two q",
                                             two=2, q=8)
                    nc.vector.tensor_tensor(
                        fc[:, :].rearrange("p (s q) -> p s q", q=8),
                        fv3[:, :, 0, :], fv3[:, :, 1, :], ALU.add)
                    fb = fc
                    fview, qq = fb[:, :].rearrange("p (s q) -> p s q", q=8), 8
                else:
                    fview, qq = fb[:, :].rearrange("p (s q) -> p s q", q=16), 16
            else:
                fview, qq = fold[:, :].rearrange("p (s q) -> p s q", q=32), 32
            for c0 in range(0, npair, 2):
                seg = slice(2 * R * c0, 2 * R * (c0 + 2))
                nc.vector.tensor_reduce(
                    red[:, seg].rearrange("p (s j) -> p s j", j=R),
                    fview[:, 2 * R * c0:2 * R * (c0 + 2)], AX.X, ALU.add)
            # red cols: [idx][kind][j]: coefA at kind 0, gnr at kind 1
            rv = red[:, :].rearrange("p (i k j) -> p i k j", k=2, j=R)
            shape = [128, R * npair]
            coefA = red[:, :].rearrange("p (i k j) -> p (i k) j", k=2, j=R)
            # strided views
            cview = rv[:, :, 0, :]          # [128, npair, R]
            gview = rv[:, :, 1, :]
            gn2 = red_p.tile(shape, FP, tag="gn2")
            g3 = gn2[:, :].rearrange("p (i j) -> p i j", j=R)
            nc.vector._custom_dve(GN2_F, out=g3, in0=gview, in1=cview,
                                  s0=1e-30)
            rg = quake(red_p, gn2[:, :], shape, "rg", FP, nr=True)
            gn = red_p.tile(shape, FP, tag="gn")
            nc.vector.tensor_tensor(gn[:, :], gn2[:, :], rg[:, :], ALU.mult)
            cosg = red_p.tile(shape, FP, tag="cosg")
            nc.scalar.activation(cosg[:, :], gn[:, :], AF.Sin,
                                 bias=halfpi[:, 0:1])
            s1t = red_p.tile(shape, FP, tag="s1t")
            nc.scalar.activation(s1t[:, :], gn[:, :], AF.Sin)
            sc = red_p.tile(shape, FP, tag="sc")
            nc.vector.tensor_tensor(sc[:, :], s1t[:, :], rg[:, :], ALU.mult)
            t9 = red_p.tile(shape, FP, tag="t9")
            nc.vector.scalar_tensor_tensor(
                t9[:, :].rearrange("p (i j) -> p i j", j=R), sc[:, :]
                .rearrange("p (i j) -> p i j", j=R), -1.0, cview,
                ALU.mult, ALU.mult)
            alpha = red_p.tile(shape, BF, tag="alpha")
            nc.vector.tensor_tensor(alpha[:, :], cosg[:, :], t9[:, :], ALU.add)
            scb = red_p.tile(shape, BF, tag="scb")
            nc.vector.tensor_copy(scb[:, :], sc[:, :])
            last = it == 2
            for idx, st in enumerate(sts):
                A, Gd = st["A"], st["Gd"]
                t1 = work.tile([128, W], BF, tag="scr1")
                eng("t1").tensor_tensor(b3(t1), b3(A),
                                        bcR(alpha[:, R * idx:R * idx + R]),
                                        ALU.mult)
                t2 = work.tile([128, W], BF, tag="scr2")
                eng("t2").tensor_tensor(b3(t2), b3(Gd),
                                        bcR(scb[:, R * idx:R * idx + R]),
                                        ALU.mult)
                An = ab_p.tile([128, W], FP if last else BF,
                               tag="agf" if last else "ag")
                nc.vector.tensor_tensor(An[:, :], t1[:, :], t2[:, :], ALU.add)
                st["A"] = An
                if not last:
                    tp = ps_t.tile([128, W], BF, tag="tp")
                    for h in (0, 1):
                        for r in range(R):
                            nc.tensor.transpose(jbh(tp, h, r), jbh(An, h, r),
                                                ident[64 * h:64 * h + 64, :])
                    AT = ab_p.tile([128, W], BF, tag="at")
                    nc.scalar.copy(AT[:, :], tp[:, :])
                    st["AT"] = AT
                else:
                    for h in (0, 1):
                        nc.sync.dma_start(
                            out_d[st["b"], h],
                            b3(An[64 * h:64 * h + 64, :]))

        all_sts = [{"b": b} for b in range(NBLK)]
        for st in all_sts:
            emit_load(st)
            emit_d0(st)
        for b0 in range(0, NBLK, INTERLEAVE):
            sts = all_sts[b0:b0 + INTERLEAVE]
            for it in range(3):
                pg = work.tile([128, 2 * W * len(sts)], BF, tag="pg")
                for idx, st in enumerate(sts):
                    st["pg"], st["pgidx"] = pg, idx
                if stagger:
                    n = len(sts)
                    for k in range(n + 1):
                        if k < n:
                            emit_factor(sts[k], it)
                        if k > 0:
                            emit_gmm(sts[k - 1], it)
                else:
                    for st in sts:
                        emit_factor(st, it)
                    for st in sts:
                        emit_gmm(st, it)
                emit_update(sts, it)
    nc.compile()
    return nc


def _get_program(**kw):
    key = tuple(sorted((k, tuple(v) if isinstance(v, (list, tuple, set, frozenset))
                        else v) for k, v in kw.items()))
    if key not in _COMPILED:
        _COMPILED[key] = build_program(**kw)
    return _COMPILED[key]


def kernel(x, w_raw, _trace=False, **bkw):
    import ml_dtypes
    from concourse.bass_utils import run_bass_kernel_spmd
    if _trace:
        _trace = _ensure_trace_hook()

    bf16 = ml_dtypes.bfloat16
    x = np.asarray(x, f32)
    w_raw = np.asarray(w_raw, f32)
    B, L, C_in, d = x.shape
    N = B * L
    w = np.exp((w_raw - f32(np.log(C_in))).astype(f32)).astype(f32)
    w = (w / w.sum(axis=0, keepdims=True)).astype(f32)

    xr = x.reshape(N, C_in, d)
    # per core: [NBLK, 2, R, i, d] -> transpose to [NBLK, 2, i, R, d]
    xcore = xr.reshape(N_CORES, NBLK, 2, R, C_in, d)
    xp = np.ascontiguousarray(xcore.transpose(0, 1, 2, 4, 3, 5)).astype(bf16)
    x0p = np.ascontiguousarray(xcore[:, :, :, :, 0, :]).astype(bf16)
    w_rep = np.ascontiguousarray(
        np.broadcast_to(w.T.reshape(1, 64, 1, 64), (2, 64, R, 64))
        .transpose(0, 3, 2, 1).reshape(128, W)).astype(bf16)
    # w_rep[p, (j, o)]: lower/upper halves identical, = w[i=p%64, o]
    w_rep = np.ascontiguousarray(
        np.tile(np.repeat(w[None, :, :], 1, axis=0), (2, 1, 1))  # (2,64,64)
        .reshape(2, 64, 1, 64).repeat(R, axis=2).reshape(2 * 64, R * 64)
        ).astype(bf16)
    ident2 = np.tile(np.eye(64, dtype=bf16), (2, 1))

    nc = _get_program(**bkw)
    in_maps = []
    for k in range(N_CORES):
        in_maps.append({
            "xp": xp[k],
            "x0p": x0p[k],
            "w_rep": w_rep,
            "ident2": ident2,
        })
    res = run_bass_kernel_spmd(nc, in_maps, core_ids=list(range(N_CORES)),
                               trace=_trace)
    # out_p: [NBLK, 2, o, j, d] per core -> rows
    outs = []
    for k in range(N_CORES):
        op = res.results[k]["out_p"]          # (NBLK, 2, 64, R, 64)
        outs.append(np.ascontiguousarray(op.transpose(0, 1, 3, 2, 4))
                    .reshape(ROWS_PER_CORE, C_OUT, d))
    out = np.concatenate(outs, axis=0)
    if _trace:
        kernel.last_exec_time_ns = res.exec_time_ns
        kernel.last_results = res
    return out.reshape(B, L, C_OUT, d).astype(f32)


# revision 7
# speedup vs baseline: 2.3877x; 1.0002x over previous
"""Trainium2 Bass kernel for nn_MfdFC (spherical weighted-Frechet-mean).

Math per row n (N=1024, 128 rows/core): w = col-softmax(w_raw); a(o) <- x0;
3 iterations of  D = <a_o, x_i>;  f = (pi/2 + arctan(-D*rr))*rr with
rr = rsqrt(1+eps-D^2) (quake rsqrt on DVE, arctan on ACT);  S = w^T。f;
G = S @ X;  c = sum_d A。G;  gn = sqrt(sum G^2 - c^2);
a <- (cos gn - sinc(gn) c) a + sinc(gn) G.

Design: all matmuls bf16 (4x PE rate), elementwise mostly bf16 (2x DVE
packed mode). 128 rows/core as 4 blocks of 32, halves stacked at PSUM
partitions 0-63 / 64-127 (PE writes upper partitions directly). All FOUR
blocks pipelined (INTERLEAVE=4); tiles read in a later phase (Gd/S/Xf/An/
AT) live in deep-rotation pools and psG is freed by back-to-back ACT ops,
so pool reuse cannot cycle against the in-order engine queues. Host pre-
transposes x / pre-replicates w so every DMA is contiguous. Iteration 0 is
specialized (a==x0: [128,16] f-chain, 2 wide matmuls for G0). The coefA /
|G|^2 reductions run as a 2-level tree: bf16 pairwise folds at DVE 2x
packed rate shrink the data 4x before the 1x-mode tensor_reduce.
"""
import math
import numpy as np

f32 = np.float32

C_IN = 64
C_OUT = 64
D_DIM = 64
ROWS_PER_CORE = 128
N_CORES = 8
R = 16
NBLK = 4
W = 64 * R
RSQ_C1 = 1.7584694439735017e-30
RSQ_C2 = -2.755803843779718e-20
HALF_PI = float(f32(math.pi / 2.0))
EPS_U = float(f32(2.0 ** -22))

_COMPILED = {}

def _register_custom_ops():
    import concourse.dve_ops as dve_ops
    from concourse.dve_ops import DveOp
    from concourse.dve_spec import (
        Spec, Src0, Src1, C0, C1, lower, maxx, _has_src1 as has_src1,
    )
    from concourse.dve_uop import DveOpSpec
    from concourse.dve_table_gen import dve_ver_for

    if "ANT_RSQ_F" in dve_ops._SUB_OPCODE_FOR_NAME:
        return {n: op for n, op in ((o.name, o) for o in dve_ops.OPS)
                if n.startswith("ANT_")}

    def _ref_rsq_f(in0, in1, s0, s1, imm2):
        u = np.asarray(in0, f32)
        nt = np.asarray(in1, f32)
        m1 = (nt * f32(s0)).astype(f32)
        m2 = (m1 * nt).astype(f32)
        m3 = (m2 * f32(s0)).astype(f32)
        t = (m3 * u).astype(f32)
        return ((t + f32(s1)) * nt).astype(f32)

    _m1 = Src1 * C0
    _m3 = (_m1 * Src1) * C0
    RSQ_F = DveOp("ANT_RSQ_F",
                  Spec(body=((_m3 * Src0) + C1) * Src1, reference=_ref_rsq_f),
                  subdim=False, uops_sha={})

    def _ref_rsq_nr(in0, in1, s0, s1, imm2):
        u = np.asarray(in0, f32); y = np.asarray(in1, f32)
        a = (u * y).astype(f32)
        b = (a * y).astype(f32)
        return ((f32(s0) - (b * f32(s1)).astype(f32)) * y).astype(f32)

    RSQ_NR = DveOp("ANT_RSQ_NR",
                   Spec(body=(C0 - ((Src0 * Src1) * Src1) * C1) * Src1,
                        reference=_ref_rsq_nr),
                   subdim=False, uops_sha={})

    def _ref_gn2(in0, in1, s0, s1, imm2):
        raw = np.asarray(in0, f32); c = np.asarray(in1, f32)
        return np.maximum((raw - (c * c).astype(f32)).astype(f32), f32(s0))

    GN2_F = DveOp("ANT_GN2_F",
                  Spec(body=maxx(Src0 - Src1 * Src1, C0), reference=_ref_gn2),
                  subdim=False, uops_sha={})

    ops = [RSQ_F, RSQ_NR, GN2_F]
    base = dve_ops._CUSTOM_DVE_ROW_BASE + len(dve_ops.OPS)
    for i, op in enumerate(ops):
        dve_ops._SUB_OPCODE_FOR_NAME[op.name] = base + i
    for trn in ("TRN2",):
        ver = dve_ver_for(trn)
        for op in ops:
            uops = lower(op.spec, ver=ver)
            s = DveOpSpec(name=op.name, opcode=dve_ops.get_dve_sub_opcode(op.name),
                          uops=uops, rd1_en=has_src1(op.spec))
            op.uops_sha[ver] = s.sha(ver)
    dve_ops.OPS.extend(ops)
    dve_ops.CUSTOM_DVE_SPECS.update({op.name: op.spec for op in ops})
    return {op.name: op for op in ops}



def _ensure_trace_hook():
    try:
        from antenv.axon_hooks import get_axon_ntff_profile_hook
        return get_axon_ntff_profile_hook() is not None
    except ImportError:
        pass
    try:
        import sys, types
        import antenv
        from trn_agent_boot.trn_boot import _ntff_profile_via_ctypes
        mod = types.ModuleType("antenv.axon_hooks")
        _h = {}
        mod.set_axon_ntff_profile_hook = lambda h: _h.__setitem__("h", h)
        mod.get_axon_ntff_profile_hook = lambda: _h.get("h")
        sys.modules["antenv.axon_hooks"] = mod
        antenv.axon_hooks = mod
        mod.set_axon_ntff_profile_hook(
            _ntff_profile_via_ctypes("/opt/axon/libaxon_pjrt.so"))
        return True
    except Exception:
        return False





def build_program(INTERLEAVE=4, gps=(), redsplit=False, wbufs=2,
                  stagger=False, psf=2, pst=2, pss=2, dbufs=None, fold2=True, fold3=False):
    from contextlib import ExitStack
    import concourse.bacc as bacc
    import concourse.mybir as mybir
    import concourse.tile as tile

    gps = frozenset(gps)
    FP = mybir.dt.float32
    BF = mybir.dt.bfloat16
    I32 = mybir.dt.int32
    AF = mybir.ActivationFunctionType
    ALU = mybir.AluOpType
    AX = mybir.AxisListType

    OPS = _register_custom_ops()
    RSQ_F, RSQ_NR, GN2_F = OPS["ANT_RSQ_F"], OPS["ANT_RSQ_NR"], OPS["ANT_GN2_F"]

    nc = bacc.Bacc()
    # x pre-transposed on host: [block, half, i, j, d]
    x_d = nc.dram_tensor("xp", (NBLK, 2, C_IN, R, D_DIM), BF,
                         kind="ExternalInput")
    # x0 rows: [block, half, j, d]
    x0_d = nc.dram_tensor("x0p", (NBLK, 2, R, D_DIM), BF, kind="ExternalInput")
    w_d = nc.dram_tensor("w_rep", (128, W), BF, kind="ExternalInput")
    id_d = nc.dram_tensor("ident2", (128, 64), BF, kind="ExternalInput")
    # output in SBUF-natural order: [block, half, o, j, d]
    out_d = nc.dram_tensor("out_p", (NBLK, 2, C_OUT, R, D_DIM), FP,
                           kind="ExternalOutput")

    ctx = ExitStack()
    with ctx:
        tc = ctx.enter_context(tile.TileContext(nc))
        const = ctx.enter_context(tc.tile_pool(name="const", bufs=1))
        xg_p = ctx.enter_context(tc.tile_pool(name="xg", bufs=NBLK))
        work = ctx.enter_context(tc.tile_pool(name="work", bufs=wbufs))
        deep = ctx.enter_context(tc.tile_pool(name="deep", bufs=dbufs or max(2, INTERLEAVE)))
        ab_p = ctx.enter_context(tc.tile_pool(name="ab", bufs=NBLK))
        red_p = ctx.enter_context(tc.tile_pool(name="red", bufs=max(2, INTERLEAVE)))
        ps_f = ctx.enter_context(tc.tile_pool(name="psf", bufs=psf, space="PSUM"))
        ps_t = ctx.enter_context(tc.tile_pool(name="pst", bufs=pst, space="PSUM"))
        ps_s = ctx.enter_context(tc.tile_pool(name="pss", bufs=pss, space="PSUM"))

        def eng(name):
            return nc.gpsimd if name in gps else nc.vector

        # ---- constants (all contiguous DMAs)
        w_g = const.tile([128, W], BF, tag="wg")
        nc.sync.dma_start(w_g[:, :], w_d[:, :])
        ident = const.tile([128, 64], BF, tag="ident")
        nc.sync.dma_start(ident[:, :], id_d[:, :])
        halfpi = const.tile([128, 1], FP, tag="halfpi")
        nc.vector.memset(halfpi[:, :], HALF_PI)

        def jbh(t, h, j):
            return t[64 * h:64 * h + 64, 64 * j:64 * j + 64]

        def b3(t):
            return t[:, :].rearrange("p (j d) -> p j d", d=64)

        def bcR(small_ap):      # [128, R] ap -> broadcast (p, j, 64)
            return small_ap.rearrange("p (j o) -> p j o", o=1) \
                .broadcast_to([128, R, 64])

        def emit_load(st):
            b = st["b"]
            X = xg_p.tile([128, W], BF, tag="xg")
            for h in (0, 1):
                nc.sync.dma_start(b3(X[64 * h:64 * h + 64, :]), x_d[b, h])
            A0 = ab_p.tile([128, W], BF, tag="a0")
            for h in (0, 1):
                nc.sync.dma_start(
                    A0[64 * h:64 * h + 64, :].rearrange("p (j d) -> p j d", d=64),
                    x0_d[b:b + 1, h].rearrange("b j d -> b j d")
                    .broadcast_to([64, R, 64]))
            tp = ps_t.tile([128, W], BF, tag="tp")
            for h in (0, 1):
                for r in range(R):
                    nc.tensor.transpose(jbh(tp, h, r), jbh(X, h, r),
                                        ident[64 * h:64 * h + 64, :])
            XT = xg_p.tile([128, W], BF, tag="xt")
            nc.scalar.copy(XT[:, :], tp[:, :])
            st["X"], st["XT"], st["A"] = X, XT, A0

        def quake(pool, src_ap, shape, tagp, out_dt, nr=False):
            seed = pool.tile(shape, FP, tag=tagp + "sd")
            nc.vector.tensor_scalar(seed[:, :].bitcast(I32),
                                    src_ap.bitcast(I32), 1, -1,
                                    ALU.logical_shift_right, ALU.bitwise_xor)
            rr = pool.tile(shape, out_dt, tag=tagp + "rr")
            nc.vector._custom_dve(RSQ_F, out=rr[:, :], in0=src_ap,
                                  in1=seed[:, :], s0=RSQ_C1, s1=RSQ_C2)
            if not nr:
                return rr
            rr2 = pool.tile(shape, out_dt, tag=tagp + "r2")
            nc.vector._custom_dve(RSQ_NR, out=rr2[:, :], in0=src_ap,
                                  in1=rr[:, :], s0=1.5, s1=0.5)
            return rr2

        # ---------- iteration 0: per-block D0 + small f-chain
        def emit_d0(st):
            XT = st["XT"]
            psD0 = ps_s.tile([128, R], FP, tag="d0")
            for h in (0, 1):
                for r in range(R):
                    nc.tensor.matmul(
                        psD0[64 * h:64 * h + 64, r:r + 1],
                        jbh(XT, h, r),
                        XT[64 * h:64 * h + 64, 64 * r:64 * r + 1])
            shape = [128, R]
            q0 = red_p.tile(shape, FP, tag="f0q")
            nc.scalar.activation(q0[:, :], psD0[:, :], AF.Square)
            u0 = red_p.tile(shape, FP, tag="f0u")
            nc.vector.tensor_scalar(u0[:, :], q0[:, :], -1.0, 1.0 + EPS_U,
                                    ALU.mult, ALU.add)
            rr0 = quake(red_p, u0[:, :], shape, "f0", FP)
            zs0 = red_p.tile(shape, FP, tag="f0z")
            nc.vector.tensor_tensor(zs0[:, :], psD0[:, :], rr0[:, :], ALU.mult)
            th0 = red_p.tile(shape, FP, tag="f0t")
            nc.scalar.activation(th0[:, :], zs0[:, :], AF.Arctan, scale=-1.0)
            f0 = red_p.tile(shape, BF, tag="f0v")
            nc.vector.scalar_tensor_tensor(f0[:, :], th0[:, :], HALF_PI,
                                           rr0[:, :], ALU.add, ALU.mult)
            st["f0"] = f0[:, :]

        def emit_factor(st, it):
            X, XT = st["X"], st["XT"]
            if it == 0:
                Xf = deep.tile([128, W], BF, tag="xf")
                nc.vector.tensor_tensor(b3(Xf), b3(X), bcR(st["f0"]), ALU.mult)
                st["Xf"] = Xf
                return
            AT = st["AT"]
            psD = ps_f.tile([128, W], FP, tag="mmf")
            for h in (0, 1):
                for r in range(R):
                    nc.tensor.matmul(jbh(psD, h, r), jbh(XT, h, r),
                                     jbh(AT, h, r))
            q = work.tile([128, W], FP, tag="ffq")
            nc.scalar.activation(q[:, :], psD[:, :], AF.Square)
            Dd = work.tile([128, W], BF, tag="dd")
            nc.scalar.copy(Dd[:, :], psD[:, :])
            u = work.tile([128, W], FP, tag="ffu")
            eng("u").tensor_scalar(u[:, :], q[:, :], -1.0, 1.0 + EPS_U,
                                   ALU.mult, ALU.add)
            rr = quake(work, u[:, :], [128, W], "ff", BF)
            zs = work.tile([128, W], BF, tag="zs")
            nc.vector.tensor_tensor(zs[:, :], Dd[:, :], rr[:, :], ALU.mult)
            th = work.tile([128, W], BF, tag="th")
            nc.scalar.activation(th[:, :], zs[:, :], AF.Arctan, scale=-1.0)
            thp = work.tile([128, W], BF, tag="thp")
            nc.vector.tensor_scalar(thp[:, :], th[:, :], HALF_PI, None,
                                    ALU.add)
            f = work.tile([128, W], BF, tag="fv")
            nc.vector.tensor_tensor(f[:, :], thp[:, :], rr[:, :], ALU.mult)
            S = deep.tile([128, W], BF, tag="sg")
            nc.vector.tensor_tensor(S[:, :], w_g[:, :], f[:, :], ALU.mult)
            st["S"] = S

        def emit_gmm(st, it):
            X = st["X"]
            psG = ps_f.tile([128, W], FP, tag="mmf")
            if it == 0:
                Xf = st["Xf"]
                for h in (0, 1):
                    for c in (0, 512):
                        nc.tensor.matmul(psG[64 * h:64 * h + 64, c:c + 512],
                                         w_g[64 * h:64 * h + 64, 0:64],
                                         Xf[64 * h:64 * h + 64, c:c + 512])
            else:
                S = st["S"]
                for h in (0, 1):
                    for r in range(R):
                        nc.tensor.matmul(jbh(psG, h, r), jbh(S, h, r),
                                         jbh(X, h, r))
            Gd = deep.tile([128, W], BF, tag="gd")
            nc.scalar.copy(Gd[:, :], psG[:, :])
            # write this block's prod/g2 slices now so psG frees in ACT order
            pg, idx = st["pg"], st["pgidx"]
            off = 2 * W * idx
            nc.scalar.activation(pg[:, off + W:off + 2 * W],
                                 psG[:, :], AF.Square)
            eng("prod").tensor_tensor(pg[:, off:off + W], st["A"][:, :],
                                      Gd[:, :], ALU.mult)
            st["Gd"] = Gd

        def emit_update(sts, it):
            npair = len(sts)
            pg = sts[0]["pg"]
            nred = 2 * R * npair
            red = red_p.tile([128, nred], FP, tag="red")
            # stage 1: pairwise fold at bf16 2x mode (halves reduce volume)
            fold = work.tile([128, W * npair], BF, tag="fold")
            pv = pg[:, :].rearrange("p (s two q) -> p s two q", two=2, q=32)
            nc.vector.tensor_tensor(
                fold[:, :].rearrange("p (s q) -> p s q", q=32),
                pv[:, :, 0, :], pv[:, :, 1, :], ALU.add)
            if fold2:
                fb = work.tile([128, W * npair // 2], BF, tag="fold2")
                fv2 = fold[:, :].rearrange("p (s two q) -> p s two q",
                                           two=2, q=16)
                nc.vector.tensor_tensor(
                    fb[:, :].rearrange("p (s q) -> p s q", q=16),
                    fv2[:, :, 0, :], fv2[:, :, 1, :], ALU.add)
                if fold3:
                    fc = work.tile([128, W * npair // 4], BF, tag="fold3")
                    fv3 = fb[:, :].rearrange("p (s two q) -> p s two q",
                                             two=2, q=8)
                    nc.vector.tensor_tensor(
                        fc[:, :].rearrange("p (s q) -> p s q", q=8),
                        fv3[:, :, 0, :], fv3[:, :, 1, :], ALU.add)
                    fb = fc
                    fview, qq = fb[:, :].rearrange("p (s q) -> p s q", q=8), 8
                else:
                    fview, qq = fb[:, :].rearrange("p (s q) -> p s q", q=16), 16
            else:
                fview, qq = fold[:, :].rearrange("p (s q) -> p s q", q=32), 32
            for c0 in range(0, npair, 2):
                seg = slice(2 * R * c0, 2 * R * (c0 + 2))
                nc.vector.tensor_reduce(
                    red[:, seg].rearrange("p (s j) -> p s j", j=R),
                    fview[:, 2 * R * c0:2 * R * (c0 + 2)], AX.X, ALU.add)
            # red cols: [idx][kind][j]: coefA at kind 0, gnr at kind 1
            rv = red[:, :].rearrange("p (i k j) -> p i k j", k=2, j=R)
            shape = [128, R * npair]
            coefA = red[:, :].rearrange("p (i k j) -> p (i k) j", k=2, j=R)
            # strided views
            cview = rv[:, :, 0, :]          # [128, npair, R]
            gview = rv[:, :, 1, :]
            gn2 = red_p.tile(shape, FP, tag="gn2")
            g3 = gn2[:, :].rearrange("p (i j) -> p i j", j=R)
            nc.vector._custom_dve(GN2_F, out=g3, in0=gview, in1=cview,
                                  s0=1e-30)
            rg = quake(red_p, gn2[:, :], shape, "rg", FP, nr=True)
            gn = red_p.tile(shape, FP, tag="gn")
            nc.vector.tensor_tensor(gn[:, :], gn2[:, :], rg[:, :], ALU.mult)
            cosg = red_p.tile(shape, FP, tag="cosg")
            nc.scalar.activation(cosg[:, :], gn[:, :], AF.Sin,
                                 bias=halfpi[:, 0:1])
            s1t = red_p.tile(shape, FP, tag="s1t")
            nc.scalar.activation(s1t[:, :], gn[:, :], AF.Sin)
            sc = red_p.tile(shape, FP, tag="sc")
            nc.vector.tensor_tensor(sc[:, :], s1t[:, :], rg[:, :], ALU.mult)
            t9 = red_p.tile(shape, FP, tag="t9")
            nc.vector.scalar_tensor_tensor(
                t9[:, :].rearrange("p (i j) -> p i j", j=R), sc[:, :]
                .rearrange("p (i j) -> p i j", j=R), -1.0, cview,
                ALU.mult, ALU.mult)
            alpha = red_p.tile(shape, BF, tag="alpha")
            nc.vector.tensor_tensor(alpha[:, :], cosg[:, :], t9[:, :], ALU.add)
            scb = red_p.tile(shape, BF, tag="scb")
            nc.vector.tensor_copy(scb[:, :], sc[:, :])
            last = it == 2
            for idx, st in enumerate(sts):
                A, Gd = st["A"], st["Gd"]
                t1 = work.tile([128, W], BF, tag="scr1")
                eng("t1").tensor_tensor(b3(t1), b3(A),
                                        bcR(alpha[:, R * idx:R * idx + R]),
                                        ALU.mult)
                t2 = work.tile([128, W], BF, tag="scr2")
                eng("t2").tensor_tensor(b3(t2), b3(Gd),
                                        bcR(scb[:, R * idx:R * idx + R]),
                                        ALU.mult)
                An = ab_p.tile([128, W], FP if last else BF,
                               tag="agf" if last else "ag")
                nc.vector.tensor_tensor(An[:, :], t1[:, :], t2[:, :], ALU.add)
                st["A"] = An
                if not last:
                    tp = ps_t.tile([128, W], BF, tag="tp")
                    for h in (0, 1):
                        for r in range(R):
                            nc.tensor.transpose(jbh(tp, h, r), jbh(An, h, r),
                                                ident[64 * h:64 * h + 64, :])
                    AT = ab_p.tile([128, W], BF, tag="at")
                    nc.scalar.copy(AT[:, :], tp[:, :])
                    st["AT"] = AT
                else:
                    for h in (0, 1):
                        nc.sync.dma_start(
                            out_d[st["b"], h],
                            b3(An[64 * h:64 * h + 64, :]))

        all_sts = [{"b": b} for b in range(NBLK)]
        for st in all_sts:
            emit_load(st)
            emit_d0(st)
        for b0 in range(0, NBLK, INTERLEAVE):
            sts = all_sts[b0:b0 + INTERLEAVE]
            for it in range(3):
                pg = work.tile([128, 2 * W * len(sts)], BF, tag="pg")
                for idx, st in enumerate(sts):
                    st["pg"], st["pgidx"] = pg, idx
                if stagger:
                    n = len(sts)
                    for k in range(n + 1):
                        if k < n:
                            emit_factor(sts[k], it)
                        if k > 0:
                            emit_gmm(sts[k - 1], it)
                else:
                    for st in sts:
                        emit_factor(st, it)
                    for st in sts:
                        emit_gmm(st, it)
                emit_update(sts, it)
    nc.compile()
    return nc


def _get_program(**kw):
    key = tuple(sorted((k, tuple(v) if isinstance(v, (list, tuple, set, frozenset))
                        else v) for k, v in kw.items()))
    if key not in _COMPILED:
        _COMPILED[key] = build_program(**kw)
    return _COMPILED[key]


def kernel(x, w_raw, _trace=False, **bkw):
    import ml_dtypes
    from concourse.bass_utils import run_bass_kernel_spmd
    if _trace:
        _trace = _ensure_trace_hook()

    bf16 = ml_dtypes.bfloat16
    x = np.asarray(x, f32)
    w_raw = np.asarray(w_raw, f32)
    B, L, C_in, d = x.shape
    N = B * L
    w = np.exp((w_raw - f32(np.log(C_in))).astype(f32)).astype(f32)
    w = (w / w.sum(axis=0, keepdims=True)).astype(f32)

    xr = x.reshape(N, C_in, d)
    # per core: [NBLK, 2, R, i, d] -> transpose to [NBLK, 2, i, R, d]
    xcore = xr.reshape(N_CORES, NBLK, 2, R, C_in, d)
    xp = np.ascontiguousarray(xcore.transpose(0, 1, 2, 4, 3, 5)).astype(bf16)
    x0p = np.ascontiguousarray(xcore[:, :, :, :, 0, :]).astype(bf16)
    w_rep = np.ascontiguousarray(
        np.broadcast_to(w.T.reshape(1, 64, 1, 64), (2, 64, R, 64))
        .transpose(0, 3, 2, 1).reshape(128, W)).astype(bf16)
    # w_rep[p, (j, o)]: lower/upper halves identical, = w[i=p%64, o]
    w_rep = np.ascontiguousarray(
        np.tile(np.repeat(w[None, :, :], 1, axis=0), (2, 1, 1))  # (2,64,64)
        .reshape(2, 64, 1, 64).repeat(R, axis=2).reshape(2 * 64, R * 64)
        ).astype(bf16)
    ident2 = np.tile(np.eye(64, dtype=bf16), (2, 1))

    nc = _get_program(**bkw)
    in_maps = []
    for k in range(N_CORES):
        in_maps.append({
            "xp": xp[k],
            "x0p": x0p[k],
            "w_rep": w_rep,
            "ident2": ident2,
        })
    res = run_bass_kernel_spmd(nc, in_maps, core_ids=list(range(N_CORES)),
                               trace=_trace)
    # out_p: [NBLK, 2, o, j, d] per core -> rows
    outs = []
    for k in range(N_CORES):
        op = res.results[k]["out_p"]          # (NBLK, 2, 64, R, 64)
        outs.append(np.ascontiguousarray(op.transpose(0, 1, 3, 2, 4))
                    .reshape(ROWS_PER_CORE, C_OUT, d))
    out = np.concatenate(outs, axis=0)
    if _trace:
        kernel.last_exec_time_ns = res.exec_time_ns
        kernel.last_results = res
    return out.reshape(B, L, C_OUT, d).astype(f32)
